# revision 7
# baseline (speedup 1.0000x reference)
"""AttentionBlock (GroupNorm + QKV 1x1 + 8-head attention + proj + residual)
as a Bass/Tile SPMD kernel for 8 Trainium2 NeuronCores.

Sharding: b*heads = 16 attention heads over 8 cores -> 2 heads/core.
GroupNorm + QKV input work is replicated within each 4-core batch group;
the attention outputs `a` are AllGathered (bf16) across the group and each
core computes the output projection for its own 128 output channels.

v2 design notes (vs the ReduceScatter baseline):
 - per-si software pipeline: QK pair (row-tiled, both heads concurrent on
   the PE array) -> exp -> AV, interleaved so the PE never idles and the
   HAM clock gate stays warm.
 - exp split across engines: head0 on ScalarE (ACT Exp), head1 on VectorE
   via the Schraudolph bit trick (y=x*a+b written as int16, reinterpreted
   as bf16).  Validated numerically: full-Schraudolph end-to-end rel err
   6e-4 (tolerance 2e-2); logits span only [-6.7, 7.1].
 - AV stationary is [v | ones] (65 cols); the ones column accumulates the
   softmax denominator in PSUM row 64.
 - softmax normalization uses reciprocal_approx_fast + a DMA broadcast
   through DRAM (per-column scale).
 - output projection: AllGather of bf16 `a` (4-core groups), then each
   core computes its 128 output channels locally; residual + folded bias
   applied with one fused DVE op.

Math rewrites (exact softmax invariances, as baseline):
 - k-bias dropped; v-bias + proj-bias folded into host cbias
 - attention scale folded into q-side weights/bias
 - no max-subtraction in softmax (logits are O(+-7), fp32-safe)
"""

import math
import os

os.environ.setdefault("JAX_PLATFORMS", "")

import ml_dtypes
import numpy as np

import concourse.bass as bass
import concourse.mybir as mybir
import concourse.tile as tile
from concourse.bass_utils import run_bass_kernel_spmd
from concourse.vector_clock import ScopedClock

F32 = mybir.dt.float32
BF16 = mybir.dt.bfloat16
I16 = mybir.dt.int16
AX = mybir.AxisListType.X
ALU = mybir.AluOpType
AF = mybir.ActivationFunctionType

B, C, H, W = 2, 512, 64, 64
L = H * W                  # 4096
HEADS = 8
CH = C // HEADS            # 64
GROUPS = 32
GPT = GROUPS // 4          # groups per 128-channel tile = 8
CPG = C // GROUPS          # channels per group = 16
EPS = 1e-6
SCALE = 1.0 / math.sqrt(math.sqrt(CH))

P = 128
KT = C // P                # 4 contraction tiles
NT = L // 512              # 8 t-blocks
NS = L // P                # 32 s-tiles
TB = 512

LAG = 2                    # AV trails QK by LAG si-steps
PJ_OFS = 16                # proj(tb) emitted at step (tb+1)*NS + PJ_OFS

# Schraudolph fast-exp for bf16 via int16 bits: bits = x*128/ln2 + (127*128-c)
EXPA = 128.0 / math.log(2.0)
EXPB = 127.0 * 128.0 - 5.5
FEXP_DVE = os.environ.get("FEXP_DVE", "0") == "1"


class SplitDrainTileContext(tile.TileContext):
    """TileContext whose final drain is split into single-wait drains (this
    toolchain's walrus rejects CTRL instructions with >1 sync wait)."""

    def _drain_and_barrier(self, tick_clock, wait_clock):
        g = tick_clock.global_clock
        entries = [(p, g[p]) for p in range(len(g)) if g[p] > 0]
        for proc, tick in entries:
            partial = ScopedClock()
            partial.require_at_least(None, proc, tick)
            d = self.nc.sync.drain()
            wait_clock.add_sem_waits(d.ins, partial)
        if not entries:
            d = self.nc.sync.drain()
            wait_clock.add_sem_waits(d.ins, ScopedClock({None: g}))
        self.nc.all_engine_barrier()
        assert self.sems is not None
        popped = self.nc._tile_sem_poison_stack.pop()
        assert popped is self._sem_poison
        self.nc.clear_and_free_semaphores(list(self.sems.allocated().values()))
        self.nc.all_engine_barrier()


def _emit(nc, tc, ctx_pools):
    """Emit the per-core program. All per-core differences come from inputs."""
    xb16 = nc.declare_dram_parameter("xb16", [KT, P, L], BF16, isOutput=False)
    xres = nc.declare_dram_parameter("xres", [P, L], F32, isOutput=False)
    wqkvT = nc.declare_dram_parameter("wqkvT", [KT, P, 384], BF16, isOutput=False)
    bq = nc.declare_dram_parameter("bq", [P, 1], F32, isOutput=False)
    wprojT4 = nc.declare_dram_parameter("wprojT4", [KT, P, P], BF16, isOutput=False)
    gamma_t = nc.declare_dram_parameter("gamma_t", [KT, P, 1], F32, isOutput=False)
    beta_t = nc.declare_dram_parameter("beta_t", [KT, P, 1], F32, isOutput=False)
    gmask = nc.declare_dram_parameter("gmask", [P, GPT], F32, isOutput=False)
    gmaskT = nc.declare_dram_parameter("gmaskT", [GPT, P], F32, isOutput=False)
    cbias = nc.declare_dram_parameter("cbias", [P, 1], F32, isOutput=False)
    ident = nc.declare_dram_parameter("ident", [P, P], BF16, isOutput=False)
    out = nc.declare_dram_parameter("out", [P, L], F32, isOutput=True)

    # ---------------- long-lived pools ----------------
    cpool = ctx_pools.enter_context(tc.tile_pool(name="consts", bufs=1))
    w_t = []
    for kt in range(KT):
        wt = cpool.tile([P, 384], BF16, name=f"w{kt}")
        nc.sync.dma_start(wt[:], wqkvT[kt])
        w_t.append(wt)
    wp_t = []
    for kt in range(KT):
        wp = cpool.tile([P, P], BF16, name=f"wp{kt}")
        nc.sync.dma_start(wp[:], wprojT4[kt])
        wp_t.append(wp)
    bq_t = cpool.tile([P, 1], F32, name="bqt")
    nc.sync.dma_start(bq_t[:], bq[:])
    gm_t = cpool.tile([P, GPT], F32, name="gmt")
    nc.sync.dma_start(gm_t[:], gmask[:])
    gmT_t = cpool.tile([GPT, P], F32, name="gmTt")
    nc.sync.dma_start(gmT_t[:], gmaskT[:])
    cb_t = cpool.tile([P, 1], F32, name="cbt")
    nc.sync.dma_start(cb_t[:], cbias[:])
    eps_t = cpool.tile([GPT, 1], F32, name="epst")
    nc.gpsimd.memset(eps_t[:], EPS)
    ident_t = cpool.tile([P, P], BF16, name="identt")
    nc.sync.dma_start(ident_t[:], ident[:])
    ga_t, be_t = [], []
    for kt in range(KT):
        g = cpool.tile([P, 1], F32, name=f"ga{kt}")
        nc.sync.dma_start(g[:], gamma_t[kt])
        ga_t.append(g)
        b = cpool.tile([P, 1], F32, name=f"be{kt}")
        nc.sync.dma_start(b[:], beta_t[kt])
        be_t.append(b)

    qkpool = ctx_pools.enter_context(tc.tile_pool(name="qk", bufs=1))
    q_both = qkpool.tile([P, L], BF16, name="q_both")
    k_both = qkpool.tile([P, L], BF16, name="k_both")
    a_both = qkpool.tile([P, L], BF16, name="a_both")
    xres_t = qkpool.tile([P, L], F32, name="xres_t")
    nc.sync.dma_start(xres_t[:], xres[:])

    vtpool = ctx_pools.enter_context(tc.tile_pool(name="vt", bufs=1))
    # [v_h0 (64) | ones | v_h1 (64) | ones]
    vt_t = [vtpool.tile([P, 130], BF16, name=f"vt{si}") for si in range(NS)]

    # prime the ACT exp table set before the main loop needs it
    prim = cpool.tile([1, 2], F32, name="prim")
    nc.gpsimd.memset(prim[:], 0.0)
    nc.scalar.activation(prim[:], prim[:], AF.Exp)

    # ---------------- phase 1: load x, GroupNorm, QKV, vT ----------------
    with tc.tile_pool(name="ph1", bufs=1) as ph1, \
         tc.tile_pool(name="ph1ps", bufs=2, space="PSUM") as ph1ps:
        xn_t = []
        for kt in range(KT):
            x_t = ph1.tile([P, L], BF16, name=f"x{kt}", tag="x", bufs=4)
            nc.sync.dma_start(x_t[:], xb16[kt])
            sums = ph1.tile([P, 2], F32, name=f"sums{kt}", tag="sums", bufs=4)
            if kt < 2:
                # ScalarE path: identity/square with free-dim accumulate
                scr = ph1.tile([P, L], BF16, name=f"scr{kt}", tag="scr", bufs=2)
                nc.scalar.activation(scr[:], x_t[:], AF.Identity,
                                     accum_out=sums[:, 0:1])
                scr2 = ph1.tile([P, L], BF16, name=f"scr2_{kt}", tag="scr2",
                                bufs=2)
                nc.scalar.activation(scr2[:], x_t[:], AF.Square,
                                     accum_out=sums[:, 1:2])
            else:
                # VectorE path
                nc.vector.tensor_reduce(sums[:, 0:1], x_t[:], AX, ALU.add)
                sq = ph1.tile([P, L], BF16, name=f"sq{kt}", tag="scr2", bufs=2)
                nc.vector.tensor_tensor(sq[:], x_t[:], x_t[:], op=ALU.mult)
                nc.vector.tensor_reduce(sums[:, 1:2], sq[:], AX, ALU.add)
            # group stats: [8, 2] = mask^T @ sums  -> [sum_x, sum_x2] per group
            gs_ps = ph1ps.tile([GPT, 2], F32, name=f"gs{kt}", tag="gs")
            nc.tensor.matmul(gs_ps[:], gm_t[:], sums[:], start=True, stop=True)
            gsm = ph1.tile([GPT, 2], F32, name=f"gsm{kt}", tag="gsm", bufs=2)
            nc.vector.tensor_scalar_mul(gsm[:], gs_ps[:], 1.0 / (CPG * L))
            var = ph1.tile([GPT, 1], F32, name=f"var{kt}", tag="var", bufs=2)
            nc.vector.tensor_tensor(var[:], gsm[:, 0:1], gsm[:, 0:1], op=ALU.mult)
            nc.vector.tensor_tensor(var[:], gsm[:, 1:2], var[:], op=ALU.subtract)
            sd = ph1.tile([GPT, 1], F32, name=f"sd{kt}", tag="sd", bufs=2)
            nc.scalar.activation(sd[:], var[:], AF.Sqrt, bias=eps_t[:])
            grp = ph1.tile([GPT, 2], F32, name=f"grp{kt}", tag="grp", bufs=2)
            nc.vector.reciprocal(grp[:, 0:1], sd[:])
            nc.vector.tensor_copy(grp[:, 1:2], gsm[:, 0:1])
            # expand group -> per-partition (rstd, mean)
            pp_ps = ph1ps.tile([P, 2], F32, name=f"pp{kt}", tag="pp")
            nc.tensor.matmul(pp_ps[:], gmT_t[:], grp[:], start=True, stop=True)
            A = ph1.tile([P, 1], F32, name=f"A{kt}", tag="A", bufs=2)
            nc.vector.tensor_tensor(A[:], pp_ps[:, 0:1], ga_t[kt][:], op=ALU.mult)
            Bt = ph1.tile([P, 1], F32, name=f"B{kt}", tag="B", bufs=2)
            nc.vector.tensor_tensor(Bt[:], pp_ps[:, 1:2], A[:], op=ALU.mult)
            nc.vector.tensor_tensor(Bt[:], be_t[kt][:], Bt[:], op=ALU.subtract)
            xn = ph1.tile([P, L], BF16, name=f"xn{kt}")
            if kt < 2:
                nc.scalar.activation(xn[:], x_t[:], AF.Identity,
                                     bias=Bt[:], scale=A[:])
            else:
                nc.vector.tensor_scalar(xn[:], x_t[:], A[:], Bt[:],
                                        op0=ALU.mult, op1=ALU.add)
            xn_t.append(xn)

        # QKV: q with folded scale+bias (ACT), k copy (DVE), v copy (DVE)
        with tc.tile_pool(name="qkvps", bufs=2, space="PSUM") as qkvps:
            v_both = ph1.tile([P, L], BF16, name="v_both")
            for j, dst in enumerate((q_both, k_both, v_both)):
                for t in range(NT):
                    ps = qkvps.tile([P, 512], F32, name=f"qkv{j}_{t}",
                                    tag="qkvps")
                    for kt in range(KT):
                        nc.tensor.matmul(
                            ps[:],
                            w_t[kt][:, j * P:(j + 1) * P],
                            xn_t[kt][:, t * 512:(t + 1) * 512],
                            start=(kt == 0), stop=(kt == KT - 1))
                    if j == 0:
                        nc.scalar.activation(dst[:, t * 512:(t + 1) * 512],
                                             ps[:], AF.Identity, bias=bq_t[:])
                    else:
                        nc.vector.tensor_copy(
                            dst[:, t * 512:(t + 1) * 512], ps[:])
            # vT tiles via PE transpose; ones cols for the denominator trick
            for si in range(NS):
                vps = qkvps.tile([P, P], BF16, name=f"vps{si}", tag="vps")
                nc.tensor.transpose(vps[:], v_both[:, si * P:(si + 1) * P],
                                    ident_t[:])
                vt = vt_t[si]
                nc.vector.memset(vt[:, 64:65], 1.0)
                nc.vector.memset(vt[:, 129:130], 1.0)
                nc.vector.tensor_copy(vt[:, 0:64], vps[:, 0:64])
                nc.scalar.copy(vt[:, 65:129], vps[:, 64:128])

    # ------- phase 2: pipelined QK -> exp -> AV, fused proj/AG/residual -----
    rgroups = [[0, 1, 2, 3], [4, 5, 6, 7]]
    N = NT * NS
    e_tiles = {}
    av_tiles = {}
    with tc.tile_pool(name="epool", bufs=1) as epool, \
         tc.tile_pool(name="qkps", bufs=2, space="PSUM") as qkps, \
         tc.tile_pool(name="avps", bufs=1, space="PSUM") as avps, \
         tc.tile_pool(name="pjps", bufs=1, space="PSUM") as pjps, \
         tc.tile_pool(name="stg", bufs=1) as stg, \
         tc.tile_pool(name="dram", bufs=1, space="DRAM") as dpool:
        ag_in = [dpool.tile([P, TB], BF16, name=f"agi{tb}") for tb in range(NT)]
        ag_out = [dpool.tile([C, TB], BF16, name=f"ago{tb}") for tb in range(NT)]
        csd = {(tb, h): dpool.tile([1, TB], F32, name=f"csd{tb}_{h}")
               for tb in range(NT) for h in range(2)}

        def emit_qk(g):
            tb, si = divmod(g, NS)
            tsl = slice(tb * TB, (tb + 1) * TB)
            qk = qkps.tile([P, 1024], F32, name=f"qk{g}", tag="qk")
            for h in range(2):
                nc.tensor.matmul(
                    qk[:, h * 512:(h + 1) * 512],
                    k_both[64 * h:64 * h + 64, si * P:(si + 1) * P],
                    q_both[64 * h:64 * h + 64, tsl],
                    start=True, stop=True)
            e_t = epool.tile([P, 1024], BF16, name=f"e{g}", tag="e", bufs=8)
            nc.scalar.activation(e_t[:, 0:512], qk[:, 0:512], AF.Exp)
            if FEXP_DVE:
                nc.vector.tensor_scalar(e_t[:, 512:1024].bitcast(I16),
                                        qk[:, 512:1024], EXPA, EXPB,
                                        op0=ALU.mult, op1=ALU.add)
            else:
                nc.scalar.activation(e_t[:, 512:1024], qk[:, 512:1024], AF.Exp)
            e_tiles[g] = e_t

        def emit_av(g):
            tb, si = divmod(g, NS)
            if si == 0:
                av_tiles[tb] = [
                    avps.tile([65, 512], F32, name=f"av{h}_{tb}", tag=f"av{h}")
                    for h in range(2)]
            e_t = e_tiles.pop(g)
            for h in range(2):
                nc.tensor.matmul(
                    av_tiles[tb][h][:],
                    vt_t[si][:, 65 * h:65 * h + 65],
                    e_t[:, h * 512:(h + 1) * 512],
                    start=(si == 0), stop=(si == NS - 1))

        def emit_norm(tb):
            tsl = slice(tb * TB, (tb + 1) * TB)
            av = av_tiles.pop(tb)
            for h in range(2):
                # 1/d on ScalarE: exp(-ln(d)); d in [~50, ~4000] so fp32-safe
                lnd = stg.tile([1, TB], F32, name=f"lnd{tb}_{h}",
                               tag=f"lnd{h}", bufs=2)
                nc.scalar.activation(lnd[:], av[h][64:65, :], AF.Ln)
                rec = stg.tile([1, TB], F32, name=f"rec{tb}_{h}",
                               tag=f"rec{h}", bufs=2)
                nc.scalar.activation(rec[:], lnd[:], AF.Exp, scale=-1.0)
                nc.sync.dma_start(csd[(tb, h)][:, :], rec[:])
                rb = stg.tile([64, TB], F32, name=f"rb{tb}_{h}",
                              tag=f"rb{h}", bufs=2)
                nc.sync.dma_start(rb[:],
                                  csd[(tb, h)][0:1, :].to_broadcast([64, TB]))
                nc.vector.tensor_tensor(
                    a_both[64 * h:64 * h + 64, tsl], av[h][0:64, :], rb[:],
                    op=ALU.mult)
            nc.sync.dma_start(ag_in[tb][:, :], a_both[:, tsl])
            nc.gpsimd.collective_compute(
                "AllGather", ALU.bypass, replica_groups=rgroups,
                ins=[ag_in[tb][:, :]], outs=[ag_out[tb][:, :]])

        def emit_proj(tb):
            tsl = slice(tb * TB, (tb + 1) * TB)
            ag_sb = stg.tile([P, KT, TB], BF16, name=f"agsb{tb}", tag="agsb",
                             bufs=2)
            for kt in range(KT):
                nc.gpsimd.dma_start(ag_sb[:, kt, :],
                                    ag_out[tb][kt * P:(kt + 1) * P, :])
            pj = pjps.tile([P, TB], F32, name=f"pj{tb}", tag="pj", bufs=2)
            for kt in range(KT):
                nc.tensor.matmul(pj[:], wp_t[kt][:], ag_sb[:, kt, :],
                                 start=(kt == 0), stop=(kt == KT - 1))
            ob = stg.tile([P, TB], F32, name=f"ob{tb}", tag="ob", bufs=2)
            nc.vector.tensor_tensor(ob[:], pj[:], xres_t[:, tsl], op=ALU.add)
            nc.vector.tensor_scalar_add(ob[:], ob[:], cb_t[:])
            nc.gpsimd.dma_start(out[:, tsl], ob[:])

        for g in range(N + NS + PJ_OFS + 1):
            if g < N:
                emit_qk(g)
            h = g - LAG
            if 0 <= h < N:
                emit_av(h)
                if h % NS == NS - 1:
                    emit_norm(h // NS)
            tb_pj, si_pj = divmod(g - PJ_OFS, NS)
            if si_pj == 0 and 1 <= tb_pj <= NT:
                emit_proj(tb_pj - 1)


def _split_waits(nc, limit=1):
    """This toolchain's walrus only encodes `limit` sync waits per
    instruction; hoist excess waits onto same-engine NOPs inserted just
    before the over-limit instruction (semantically a stricter stall)."""
    n_split = 0
    for f in nc.m.functions:
        for bb in f.blocks:
            live = bb.instructions
            new_list = []
            changed = False
            for inst in live:
                si = inst.sync_info
                if si is not None and len(si.on_wait) > limit:
                    waits = list(si.on_wait)
                    extra, keep = waits[:-limit], waits[-limit:]
                    for j in range(0, len(extra), limit):
                        nop = mybir.InstNoOp(
                            name=f"I-wsplit-{nc.next_id()}", ins=[], outs=[])
                        nop.engine = inst.engine
                        nop.sync_info = mybir.SyncInfo(
                            on_wait=extra[j:j + limit], on_update=[])
                        new_list.append(nop)
                        n_split += 1
                    inst.sync_info = mybir.SyncInfo(
                        on_wait=keep, on_update=list(si.on_update))
                    changed = True
                new_list.append(inst)
            if changed:
                live.clear()
                live.extend(new_list)
    return n_split


_CACHE = {}


def _build():
    key = "nc"
    if key not in _CACHE:
        from contextlib import ExitStack
        nc = bass.Bass("TRN2", target_bir_lowering=False, debug=False,
                       num_devices=8)
        with SplitDrainTileContext(nc) as tc:
            with ExitStack() as pools:
                _emit(nc, tc, pools)
        _split_waits(nc)
        _CACHE[key] = nc
    return _CACHE[key]


def _host_inputs(x, gamma, beta, w_qkv, b_qkv, w_proj, b_proj):
    """Build the 8 per-core input maps."""
    xr = x.reshape(B, C, L)
    b_v = np.concatenate([b_qkv[192 * h + 128:192 * h + 192]
                          for h in range(HEADS)])
    cbias_full = w_proj @ b_v + b_proj  # [C]
    gmask = np.zeros((P, GPT), np.float32)
    for p in range(P):
        gmask[p, p // CPG] = 1.0
    xb16 = [np.ascontiguousarray(xr[b].reshape(KT, P, L)).astype(
        ml_dtypes.bfloat16) for b in range(B)]
    in_maps = []
    for core in range(8):
        b, r = divmod(core, 4)
        h0, h1 = 2 * r, 2 * r + 1
        qrows = np.concatenate([192 * h + np.arange(CH) for h in (h0, h1)])
        krows = qrows + CH
        vrows = krows + CH
        wsel = np.concatenate([w_qkv[qrows] * SCALE, w_qkv[krows] * SCALE,
                               w_qkv[vrows]], axis=0)  # [384, C]
        wpT = np.ascontiguousarray(
            w_proj[r * P:(r + 1) * P, :].T).reshape(KT, P, P)
        in_maps.append({
            "xb16": xb16[b],
            "xres": np.ascontiguousarray(xr[b, r * P:(r + 1) * P]),
            "wqkvT": np.ascontiguousarray(
                wsel.T.reshape(KT, P, 384)).astype(ml_dtypes.bfloat16),
            "bq": np.ascontiguousarray(
                (b_qkv[qrows] * SCALE).reshape(P, 1)),
            "wprojT4": wpT.astype(ml_dtypes.bfloat16),
            "gamma_t": np.ascontiguousarray(gamma.reshape(KT, P, 1)),
            "beta_t": np.ascontiguousarray(beta.reshape(KT, P, 1)),
            "gmask": gmask,
            "gmaskT": np.ascontiguousarray(gmask.T),
            "cbias": np.ascontiguousarray(
                cbias_full[r * P:(r + 1) * P].reshape(P, 1)),
            "ident": np.eye(P, dtype=ml_dtypes.bfloat16),
        })
    return in_maps


def kernel(x, gamma, beta, w_qkv, b_qkv, w_proj, b_proj, _trace=False):
    x = np.asarray(x, np.float32)
    gamma = np.asarray(gamma, np.float32)
    beta = np.asarray(beta, np.float32)
    w_qkv = np.asarray(w_qkv, np.float32)
    b_qkv = np.asarray(b_qkv, np.float32)
    w_proj = np.asarray(w_proj, np.float32)
    b_proj = np.asarray(b_proj, np.float32)

    nc = _build()
    in_maps = _host_inputs(x, gamma, beta, w_qkv, b_qkv, w_proj, b_proj)
    res = run_bass_kernel_spmd(nc, in_maps, list(range(8)), trace=_trace)
    out = np.empty((B, C, L), np.float32)
    for core in range(8):
        b, r = divmod(core, 4)
        out[b, r * P:(r + 1) * P] = res.results[core]["out"]
    if _trace:
        kernel.last_results = res
    return out.reshape(B, C, H, W)


# revision 10
# speedup vs baseline: 1.0533x; 1.0533x over previous
"""AttentionBlock (GroupNorm + QKV 1x1 + 8-head attention + proj + residual)
as a Bass/Tile SPMD kernel for 8 Trainium2 NeuronCores.

Sharding: b*heads = 16 attention heads over 8 cores -> 2 heads/core.
GroupNorm + QKV input work is replicated within each 4-core batch group;
the attention outputs `a` are AllGathered (bf16) across the group and each
core computes the output projection for its own 128 output channels.

v2 design notes (vs the ReduceScatter baseline):
 - per-si software pipeline: QK pair (row-tiled, both heads concurrent on
   the PE array) -> exp -> AV, interleaved so the PE never idles and the
   HAM clock gate stays warm.
 - exp split across engines: head0 on ScalarE (ACT Exp), head1 on VectorE
   via the Schraudolph bit trick (y=x*a+b written as int16, reinterpreted
   as bf16).  Validated numerically: full-Schraudolph end-to-end rel err
   6e-4 (tolerance 2e-2); logits span only [-6.7, 7.1].
 - AV stationary is [v | ones] (65 cols); the ones column accumulates the
   softmax denominator in PSUM row 64.
 - softmax normalization uses reciprocal_approx_fast + a DMA broadcast
   through DRAM (per-column scale).
 - output projection: AllGather of bf16 `a` (4-core groups), then each
   core computes its 128 output channels locally; residual + folded bias
   applied with one fused DVE op.

Math rewrites (exact softmax invariances, as baseline):
 - k-bias dropped; v-bias + proj-bias folded into host cbias
 - attention scale folded into q-side weights/bias
 - no max-subtraction in softmax (logits are O(+-7), fp32-safe)
"""

import math
import os

os.environ.setdefault("JAX_PLATFORMS", "")

import ml_dtypes
import numpy as np

import concourse.bass as bass
import concourse.mybir as mybir
import concourse.tile as tile
from concourse.bass_utils import run_bass_kernel_spmd
from concourse.vector_clock import ScopedClock

F32 = mybir.dt.float32
BF16 = mybir.dt.bfloat16
I16 = mybir.dt.int16
AX = mybir.AxisListType.X
ALU = mybir.AluOpType
AF = mybir.ActivationFunctionType

B, C, H, W = 2, 512, 64, 64
L = H * W                  # 4096
HEADS = 8
CH = C // HEADS            # 64
GROUPS = 32
GPT = GROUPS // 4          # groups per 128-channel tile = 8
CPG = C // GROUPS          # channels per group = 16
EPS = 1e-6
SCALE = 1.0 / math.sqrt(math.sqrt(CH))

P = 128
KT = C // P                # 4 contraction tiles
NT = L // 512              # 8 t-blocks
NS = L // P                # 32 s-tiles
TB = 512

LAG = 2                    # AV trails QK by LAG si-steps
PJ_OFS = 16                # proj(tb) emitted at step (tb+1)*NS + PJ_OFS

# Schraudolph fast-exp for bf16 via int16 bits: bits = x*128/ln2 + (127*128-c)
EXPA = 128.0 / math.log(2.0)
EXPB = 127.0 * 128.0 - 5.5
FEXP_DVE = os.environ.get("FEXP_DVE", "1") == "1"


class SplitDrainTileContext(tile.TileContext):
    """TileContext whose final drain is split into single-wait drains (this
    toolchain's walrus rejects CTRL instructions with >1 sync wait)."""

    def _drain_and_barrier(self, tick_clock, wait_clock):
        g = tick_clock.global_clock
        entries = [(p, g[p]) for p in range(len(g)) if g[p] > 0]
        for proc, tick in entries:
            partial = ScopedClock()
            partial.require_at_least(None, proc, tick)
            d = self.nc.sync.drain()
            wait_clock.add_sem_waits(d.ins, partial)
        if not entries:
            d = self.nc.sync.drain()
            wait_clock.add_sem_waits(d.ins, ScopedClock({None: g}))
        self.nc.all_engine_barrier()
        assert self.sems is not None
        popped = self.nc._tile_sem_poison_stack.pop()
        assert popped is self._sem_poison
        self.nc.clear_and_free_semaphores(list(self.sems.allocated().values()))
        self.nc.all_engine_barrier()


def _emit(nc, tc, ctx_pools):
    """Emit the per-core program. All per-core differences come from inputs."""
    xb16 = nc.declare_dram_parameter("xb16", [KT, P, L], BF16, isOutput=False)
    xres = nc.declare_dram_parameter("xres", [P, L], F32, isOutput=False)
    wqkvT = nc.declare_dram_parameter("wqkvT", [KT, P, 384], BF16, isOutput=False)
    bq = nc.declare_dram_parameter("bq", [P, 1], F32, isOutput=False)
    wprojT4 = nc.declare_dram_parameter("wprojT4", [KT, P, P], BF16, isOutput=False)
    gamma_t = nc.declare_dram_parameter("gamma_t", [KT, P, 1], F32, isOutput=False)
    beta_t = nc.declare_dram_parameter("beta_t", [KT, P, 1], F32, isOutput=False)
    gmask = nc.declare_dram_parameter("gmask", [P, GPT], F32, isOutput=False)
    gmaskT = nc.declare_dram_parameter("gmaskT", [GPT, P], F32, isOutput=False)
    cbias = nc.declare_dram_parameter("cbias", [P, 1], F32, isOutput=False)
    ident = nc.declare_dram_parameter("ident", [P, P], BF16, isOutput=False)
    out = nc.declare_dram_parameter("out", [P, L], F32, isOutput=True)

    # ---------------- long-lived pools ----------------
    cpool = ctx_pools.enter_context(tc.tile_pool(name="consts", bufs=1))
    w_t = []
    for kt in range(KT):
        wt = cpool.tile([P, 384], BF16, name=f"w{kt}")
        nc.sync.dma_start(wt[:], wqkvT[kt])
        w_t.append(wt)
    wp_t = []
    for kt in range(KT):
        wp = cpool.tile([P, P], BF16, name=f"wp{kt}")
        nc.sync.dma_start(wp[:], wprojT4[kt])
        wp_t.append(wp)
    bq_t = cpool.tile([P, 1], F32, name="bqt")
    nc.sync.dma_start(bq_t[:], bq[:])
    gm_t = cpool.tile([P, GPT], F32, name="gmt")
    nc.sync.dma_start(gm_t[:], gmask[:])
    gmT_t = cpool.tile([GPT, P], F32, name="gmTt")
    nc.sync.dma_start(gmT_t[:], gmaskT[:])
    cb_t = cpool.tile([P, 1], F32, name="cbt")
    nc.sync.dma_start(cb_t[:], cbias[:])
    eps_t = cpool.tile([GPT, 1], F32, name="epst")
    nc.gpsimd.memset(eps_t[:], EPS)
    ident_t = cpool.tile([P, P], BF16, name="identt")
    nc.sync.dma_start(ident_t[:], ident[:])
    ga_t, be_t = [], []
    for kt in range(KT):
        g = cpool.tile([P, 1], F32, name=f"ga{kt}")
        nc.sync.dma_start(g[:], gamma_t[kt])
        ga_t.append(g)
        b = cpool.tile([P, 1], F32, name=f"be{kt}")
        nc.sync.dma_start(b[:], beta_t[kt])
        be_t.append(b)

    qkpool = ctx_pools.enter_context(tc.tile_pool(name="qk", bufs=1))
    q_both = qkpool.tile([P, L], BF16, name="q_both")
    k_both = qkpool.tile([P, L], BF16, name="k_both")
    a_both = qkpool.tile([P, L], BF16, name="a_both")
    xres_t = qkpool.tile([P, L], F32, name="xres_t")
    nc.sync.dma_start(xres_t[:], xres[:])

    vtpool = ctx_pools.enter_context(tc.tile_pool(name="vt", bufs=1))
    # [v_h0 (64) | ones | v_h1 (64) | ones]
    vt_t = [vtpool.tile([P, 130], BF16, name=f"vt{si}") for si in range(NS)]

    # prime the ACT exp table set before the main loop needs it
    prim = cpool.tile([1, 2], F32, name="prim")
    nc.gpsimd.memset(prim[:], 0.0)
    nc.scalar.activation(prim[:], prim[:], AF.Exp)

    # ---------------- phase 1: load x, GroupNorm, QKV, vT ----------------
    with tc.tile_pool(name="ph1", bufs=1) as ph1, \
         tc.tile_pool(name="ph1ps", bufs=2, space="PSUM") as ph1ps:
        xn_t = []
        for kt in range(KT):
            x_t = ph1.tile([P, L], BF16, name=f"x{kt}", tag="x", bufs=4)
            nc.sync.dma_start(x_t[:], xb16[kt])
            sums = ph1.tile([P, 2], F32, name=f"sums{kt}", tag="sums", bufs=4)
            if kt < 2:
                # ScalarE path: identity/square with free-dim accumulate
                scr = ph1.tile([P, L], BF16, name=f"scr{kt}", tag="scr", bufs=2)
                nc.scalar.activation(scr[:], x_t[:], AF.Identity,
                                     accum_out=sums[:, 0:1])
                scr2 = ph1.tile([P, L], BF16, name=f"scr2_{kt}", tag="scr2",
                                bufs=2)
                nc.scalar.activation(scr2[:], x_t[:], AF.Square,
                                     accum_out=sums[:, 1:2])
            else:
                # VectorE path
                nc.vector.tensor_reduce(sums[:, 0:1], x_t[:], AX, ALU.add)
                sq = ph1.tile([P, L], BF16, name=f"sq{kt}", tag="scr2", bufs=2)
                nc.vector.tensor_tensor(sq[:], x_t[:], x_t[:], op=ALU.mult)
                nc.vector.tensor_reduce(sums[:, 1:2], sq[:], AX, ALU.add)
            # group stats: [8, 2] = mask^T @ sums  -> [sum_x, sum_x2] per group
            gs_ps = ph1ps.tile([GPT, 2], F32, name=f"gs{kt}", tag="gs")
            nc.tensor.matmul(gs_ps[:], gm_t[:], sums[:], start=True, stop=True)
            gsm = ph1.tile([GPT, 2], F32, name=f"gsm{kt}", tag="gsm", bufs=2)
            nc.vector.tensor_scalar_mul(gsm[:], gs_ps[:], 1.0 / (CPG * L))
            var = ph1.tile([GPT, 1], F32, name=f"var{kt}", tag="var", bufs=2)
            nc.vector.tensor_tensor(var[:], gsm[:, 0:1], gsm[:, 0:1], op=ALU.mult)
            nc.vector.tensor_tensor(var[:], gsm[:, 1:2], var[:], op=ALU.subtract)
            sd = ph1.tile([GPT, 1], F32, name=f"sd{kt}", tag="sd", bufs=2)
            nc.scalar.activation(sd[:], var[:], AF.Sqrt, bias=eps_t[:])
            grp = ph1.tile([GPT, 2], F32, name=f"grp{kt}", tag="grp", bufs=2)
            nc.vector.reciprocal(grp[:, 0:1], sd[:])
            nc.vector.tensor_copy(grp[:, 1:2], gsm[:, 0:1])
            # expand group -> per-partition (rstd, mean)
            pp_ps = ph1ps.tile([P, 2], F32, name=f"pp{kt}", tag="pp")
            nc.tensor.matmul(pp_ps[:], gmT_t[:], grp[:], start=True, stop=True)
            A = ph1.tile([P, 1], F32, name=f"A{kt}", tag="A", bufs=2)
            nc.vector.tensor_tensor(A[:], pp_ps[:, 0:1], ga_t[kt][:], op=ALU.mult)
            Bt = ph1.tile([P, 1], F32, name=f"B{kt}", tag="B", bufs=2)
            nc.vector.tensor_tensor(Bt[:], pp_ps[:, 1:2], A[:], op=ALU.mult)
            nc.vector.tensor_tensor(Bt[:], be_t[kt][:], Bt[:], op=ALU.subtract)
            xn = ph1.tile([P, L], BF16, name=f"xn{kt}")
            if kt < 2:
                nc.scalar.activation(xn[:], x_t[:], AF.Identity,
                                     bias=Bt[:], scale=A[:])
            else:
                nc.vector.tensor_scalar(xn[:], x_t[:], A[:], Bt[:],
                                        op0=ALU.mult, op1=ALU.add)
            xn_t.append(xn)

        # QKV: q with folded scale+bias (ACT), k copy (DVE), v copy (DVE)
        with tc.tile_pool(name="qkvps", bufs=2, space="PSUM") as qkvps:
            v_both = ph1.tile([P, L], BF16, name="v_both")
            for j, dst in enumerate((q_both, k_both, v_both)):
                for t in range(NT):
                    ps = qkvps.tile([P, 512], F32, name=f"qkv{j}_{t}",
                                    tag="qkvps")
                    for kt in range(KT):
                        nc.tensor.matmul(
                            ps[:],
                            w_t[kt][:, j * P:(j + 1) * P],
                            xn_t[kt][:, t * 512:(t + 1) * 512],
                            start=(kt == 0), stop=(kt == KT - 1))
                    if j == 0:
                        nc.scalar.activation(dst[:, t * 512:(t + 1) * 512],
                                             ps[:], AF.Identity, bias=bq_t[:])
                    else:
                        nc.vector.tensor_copy(
                            dst[:, t * 512:(t + 1) * 512], ps[:])
            # vT tiles via PE transpose; ones cols for the denominator trick
            for si in range(NS):
                vps = qkvps.tile([P, P], BF16, name=f"vps{si}", tag="vps")
                nc.tensor.transpose(vps[:], v_both[:, si * P:(si + 1) * P],
                                    ident_t[:])
                vt = vt_t[si]
                nc.vector.memset(vt[:, 64:65], 1.0)
                nc.vector.memset(vt[:, 129:130], 1.0)
                nc.vector.tensor_copy(vt[:, 0:64], vps[:, 0:64])
                nc.scalar.copy(vt[:, 65:129], vps[:, 64:128])

    # ------- phase 2: pipelined QK -> exp -> AV, fused proj/AG/residual -----
    rgroups = [[0, 1, 2, 3], [4, 5, 6, 7]]
    N = NT * NS
    e_tiles = {}
    av_tiles = {}
    with tc.tile_pool(name="epool", bufs=1) as epool, \
         tc.tile_pool(name="qkps", bufs=2, space="PSUM") as qkps, \
         tc.tile_pool(name="avps", bufs=1, space="PSUM") as avps, \
         tc.tile_pool(name="pjps", bufs=1, space="PSUM") as pjps, \
         tc.tile_pool(name="stg", bufs=1) as stg, \
         tc.tile_pool(name="dram", bufs=1, space="DRAM") as dpool:
        ag_in = [dpool.tile([P, TB], BF16, name=f"agi{tb}") for tb in range(NT)]
        ag_out = [dpool.tile([C, TB], BF16, name=f"ago{tb}") for tb in range(NT)]
        csd = {(tb, h): dpool.tile([1, TB], F32, name=f"csd{tb}_{h}")
               for tb in range(NT) for h in range(2)}

        def emit_qk(g):
            tb, si = divmod(g, NS)
            tsl = slice(tb * TB, (tb + 1) * TB)
            qk = qkps.tile([P, 1024], F32, name=f"qk{g}", tag="qk")
            for h in range(2):
                nc.tensor.matmul(
                    qk[:, h * 512:(h + 1) * 512],
                    k_both[64 * h:64 * h + 64, si * P:(si + 1) * P],
                    q_both[64 * h:64 * h + 64, tsl],
                    start=True, stop=True)
            e0 = epool.tile([P, 512], BF16, name=f"e0_{g}", tag="e0", bufs=8)
            e1 = epool.tile([P, 512], BF16, name=f"e1_{g}", tag="e1", bufs=8)
            if FEXP_DVE:
                # whole-tile bitcast only (offset-0): sliced bitcast hangs HW
                if g % 4 == 3:
                    nc.vector.tensor_scalar(e0[:].bitcast(I16), qk[:, 0:512],
                                            EXPA, EXPB,
                                            op0=ALU.mult, op1=ALU.add)
                else:
                    nc.scalar.activation(e0[:], qk[:, 0:512], AF.Exp)
                nc.vector.tensor_scalar(e1[:].bitcast(I16), qk[:, 512:1024],
                                        EXPA, EXPB,
                                        op0=ALU.mult, op1=ALU.add)
            else:
                nc.scalar.activation(e0[:], qk[:, 0:512], AF.Exp)
                nc.scalar.activation(e1[:], qk[:, 512:1024], AF.Exp)
            e_tiles[g] = (e0, e1)

        def emit_av(g):
            tb, si = divmod(g, NS)
            if si == 0:
                av_tiles[tb] = [
                    avps.tile([65, 512], F32, name=f"av{h}_{tb}", tag=f"av{h}")
                    for h in range(2)]
            e_pair = e_tiles.pop(g)
            for h in range(2):
                nc.tensor.matmul(
                    av_tiles[tb][h][:],
                    vt_t[si][:, 65 * h:65 * h + 65],
                    e_pair[h][:],
                    start=(si == 0), stop=(si == NS - 1))

        def emit_norm(tb):
            tsl = slice(tb * TB, (tb + 1) * TB)
            av = av_tiles.pop(tb)
            for h in range(2):
                # 1/d on ScalarE: exp(-ln(d)); d in [~50, ~4000] so fp32-safe
                lnd = stg.tile([1, TB], F32, name=f"lnd{tb}_{h}",
                               tag=f"lnd{h}", bufs=2)
                nc.scalar.activation(lnd[:], av[h][64:65, :], AF.Ln)
                rec = stg.tile([1, TB], F32, name=f"rec{tb}_{h}",
                               tag=f"rec{h}", bufs=2)
                nc.scalar.activation(rec[:], lnd[:], AF.Exp, scale=-1.0)
                nc.sync.dma_start(csd[(tb, h)][:, :], rec[:])
                rb = stg.tile([64, TB], F32, name=f"rb{tb}_{h}",
                              tag=f"rb{h}", bufs=2)
                nc.sync.dma_start(rb[:],
                                  csd[(tb, h)][0:1, :].to_broadcast([64, TB]))
                nc.vector.tensor_tensor(
                    a_both[64 * h:64 * h + 64, tsl], av[h][0:64, :], rb[:],
                    op=ALU.mult)
            nc.sync.dma_start(ag_in[tb][:, :], a_both[:, tsl])
            nc.gpsimd.collective_compute(
                "AllGather", ALU.bypass, replica_groups=rgroups,
                ins=[ag_in[tb][:, :]], outs=[ag_out[tb][:, :]])

        def emit_proj(tb):
            tsl = slice(tb * TB, (tb + 1) * TB)
            ag_sb = stg.tile([P, KT, TB], BF16, name=f"agsb{tb}", tag="agsb",
                             bufs=2)
            for kt in range(KT):
                nc.gpsimd.dma_start(ag_sb[:, kt, :],
                                    ag_out[tb][kt * P:(kt + 1) * P, :])
            pj = pjps.tile([P, TB], F32, name=f"pj{tb}", tag="pj", bufs=2)
            for kt in range(KT):
                nc.tensor.matmul(pj[:], wp_t[kt][:], ag_sb[:, kt, :],
                                 start=(kt == 0), stop=(kt == KT - 1))
            ob = stg.tile([P, TB], F32, name=f"ob{tb}", tag="ob", bufs=2)
            nc.vector.tensor_tensor(ob[:], pj[:], xres_t[:, tsl], op=ALU.add)
            nc.vector.tensor_scalar_add(ob[:], ob[:], cb_t[:])
            nc.gpsimd.dma_start(out[:, tsl], ob[:])

        for g in range(N + NS + PJ_OFS + 1):
            if g < N:
                emit_qk(g)
            h = g - LAG
            if 0 <= h < N:
                emit_av(h)
                if h % NS == NS - 1:
                    emit_norm(h // NS)
            tb_pj, si_pj = divmod(g - PJ_OFS, NS)
            if si_pj == 0 and 1 <= tb_pj <= NT:
                emit_proj(tb_pj - 1)


def _split_waits(nc, limit=1):
    """This toolchain's walrus only encodes `limit` sync waits per
    instruction; hoist excess waits onto same-engine NOPs inserted just
    before the over-limit instruction (semantically a stricter stall)."""
    n_split = 0
    for f in nc.m.functions:
        for bb in f.blocks:
            live = bb.instructions
            new_list = []
            changed = False
            for inst in live:
                si = inst.sync_info
                if si is not None and len(si.on_wait) > limit:
                    waits = list(si.on_wait)
                    extra, keep = waits[:-limit], waits[-limit:]
                    for j in range(0, len(extra), limit):
                        nop = mybir.InstNoOp(
                            name=f"I-wsplit-{nc.next_id()}", ins=[], outs=[])
                        nop.engine = inst.engine
                        nop.sync_info = mybir.SyncInfo(
                            on_wait=extra[j:j + limit], on_update=[])
                        new_list.append(nop)
                        n_split += 1
                    inst.sync_info = mybir.SyncInfo(
                        on_wait=keep, on_update=list(si.on_update))
                    changed = True
                new_list.append(inst)
            if changed:
                live.clear()
                live.extend(new_list)
    return n_split


_CACHE = {}


def _build():
    key = "nc"
    if key not in _CACHE:
        from contextlib import ExitStack
        nc = bass.Bass("TRN2", target_bir_lowering=False, debug=False,
                       num_devices=8)
        with SplitDrainTileContext(nc) as tc:
            with ExitStack() as pools:
                _emit(nc, tc, pools)
        _split_waits(nc)
        _CACHE[key] = nc
    return _CACHE[key]


def _host_inputs(x, gamma, beta, w_qkv, b_qkv, w_proj, b_proj):
    """Build the 8 per-core input maps."""
    xr = x.reshape(B, C, L)
    b_v = np.concatenate([b_qkv[192 * h + 128:192 * h + 192]
                          for h in range(HEADS)])
    cbias_full = w_proj @ b_v + b_proj  # [C]
    gmask = np.zeros((P, GPT), np.float32)
    for p in range(P):
        gmask[p, p // CPG] = 1.0
    xb16 = [np.ascontiguousarray(xr[b].reshape(KT, P, L)).astype(
        ml_dtypes.bfloat16) for b in range(B)]
    in_maps = []
    for core in range(8):
        b, r = divmod(core, 4)
        h0, h1 = 2 * r, 2 * r + 1
        qrows = np.concatenate([192 * h + np.arange(CH) for h in (h0, h1)])
        krows = qrows + CH
        vrows = krows + CH
        wsel = np.concatenate([w_qkv[qrows] * SCALE, w_qkv[krows] * SCALE,
                               w_qkv[vrows]], axis=0)  # [384, C]
        wpT = np.ascontiguousarray(
            w_proj[r * P:(r + 1) * P, :].T).reshape(KT, P, P)
        in_maps.append({
            "xb16": xb16[b],
            "xres": np.ascontiguousarray(xr[b, r * P:(r + 1) * P]),
            "wqkvT": np.ascontiguousarray(
                wsel.T.reshape(KT, P, 384)).astype(ml_dtypes.bfloat16),
            "bq": np.ascontiguousarray(
                (b_qkv[qrows] * SCALE).reshape(P, 1)),
            "wprojT4": wpT.astype(ml_dtypes.bfloat16),
            "gamma_t": np.ascontiguousarray(gamma.reshape(KT, P, 1)),
            "beta_t": np.ascontiguousarray(beta.reshape(KT, P, 1)),
            "gmask": gmask,
            "gmaskT": np.ascontiguousarray(gmask.T),
            "cbias": np.ascontiguousarray(
                cbias_full[r * P:(r + 1) * P].reshape(P, 1)),
            "ident": np.eye(P, dtype=ml_dtypes.bfloat16),
        })
    return in_maps


def kernel(x, gamma, beta, w_qkv, b_qkv, w_proj, b_proj, _trace=False):
    x = np.asarray(x, np.float32)
    gamma = np.asarray(gamma, np.float32)
    beta = np.asarray(beta, np.float32)
    w_qkv = np.asarray(w_qkv, np.float32)
    b_qkv = np.asarray(b_qkv, np.float32)
    w_proj = np.asarray(w_proj, np.float32)
    b_proj = np.asarray(b_proj, np.float32)

    nc = _build()
    in_maps = _host_inputs(x, gamma, beta, w_qkv, b_qkv, w_proj, b_proj)
    res = run_bass_kernel_spmd(nc, in_maps, list(range(8)), trace=_trace)
    out = np.empty((B, C, L), np.float32)
    for core in range(8):
        b, r = divmod(core, 4)
        out[b, r * P:(r + 1) * P] = res.results[core]["out"]
    if _trace:
        kernel.last_results = res
    return out.reshape(B, C, H, W)


# revision 20
# speedup vs baseline: 1.0665x; 1.0126x over previous
"""AttentionBlock (GroupNorm + QKV 1x1 + 8-head attention + proj + residual)
as a Bass/Tile SPMD kernel for 8 Trainium2 NeuronCores.

Sharding: b*heads = 16 attention heads over 8 cores -> 2 heads/core.
GroupNorm + QKV input work is replicated within each 4-core batch group;
the attention outputs `a` are AllGathered (bf16) across the group and each
core computes the output projection for its own 128 output channels.

v2 design notes (vs the ReduceScatter baseline):
 - per-si software pipeline: QK pair (row-tiled, both heads concurrent on
   the PE array) -> exp -> AV, interleaved so the PE never idles and the
   HAM clock gate stays warm.
 - exp split across engines: head0 on ScalarE (ACT Exp), head1 on VectorE
   via the Schraudolph bit trick (y=x*a+b written as int16, reinterpreted
   as bf16).  Validated numerically: full-Schraudolph end-to-end rel err
   6e-4 (tolerance 2e-2); logits span only [-6.7, 7.1].
 - AV stationary is [v | ones] (65 cols); the ones column accumulates the
   softmax denominator in PSUM row 64.
 - softmax normalization uses reciprocal_approx_fast + a DMA broadcast
   through DRAM (per-column scale).
 - output projection: AllGather of bf16 `a` (4-core groups), then each
   core computes its 128 output channels locally; residual + folded bias
   applied with one fused DVE op.

Math rewrites (exact softmax invariances, as baseline):
 - k-bias dropped; v-bias + proj-bias folded into host cbias
 - attention scale folded into q-side weights/bias
 - no max-subtraction in softmax (logits are O(+-7), fp32-safe)
"""

import math
import os

os.environ.setdefault("JAX_PLATFORMS", "")

import ml_dtypes
import numpy as np

import concourse.bass as bass
import concourse.mybir as mybir
import concourse.tile as tile
from concourse.bass_utils import run_bass_kernel_spmd
from concourse.vector_clock import ScopedClock

F32 = mybir.dt.float32
BF16 = mybir.dt.bfloat16
I16 = mybir.dt.int16
AX = mybir.AxisListType.X
ALU = mybir.AluOpType
AF = mybir.ActivationFunctionType

B, C, H, W = 2, 512, 64, 64
L = H * W                  # 4096
HEADS = 8
CH = C // HEADS            # 64
GROUPS = 32
GPT = GROUPS // 4          # groups per 128-channel tile = 8
CPG = C // GROUPS          # channels per group = 16
EPS = 1e-6
SCALE = 1.0 / math.sqrt(math.sqrt(CH))

P = 128
KT = C // P                # 4 contraction tiles
NT = L // 512              # 8 t-blocks
NS = L // P                # 32 s-tiles
TB = 512

LAG = 2                    # AV trails QK by LAG si-steps
PJ_OFS = 18                # proj(tb) emitted at step (tb+1)*NS + PJ_OFS

# Schraudolph fast-exp for bf16 via int16 bits: bits = x*128/ln2 + (127*128-c)
EXPA = 128.0 / math.log(2.0)
EXPB = 127.0 * 128.0 - 5.5
FEXP_DVE = os.environ.get("FEXP_DVE", "1") == "1"


class SplitDrainTileContext(tile.TileContext):
    """TileContext whose final drain is split into single-wait drains (this
    toolchain's walrus rejects CTRL instructions with >1 sync wait)."""

    def _drain_and_barrier(self, tick_clock, wait_clock):
        g = tick_clock.global_clock
        entries = [(p, g[p]) for p in range(len(g)) if g[p] > 0]
        for proc, tick in entries:
            partial = ScopedClock()
            partial.require_at_least(None, proc, tick)
            d = self.nc.sync.drain()
            wait_clock.add_sem_waits(d.ins, partial)
        if not entries:
            d = self.nc.sync.drain()
            wait_clock.add_sem_waits(d.ins, ScopedClock({None: g}))
        self.nc.all_engine_barrier()
        assert self.sems is not None
        popped = self.nc._tile_sem_poison_stack.pop()
        assert popped is self._sem_poison
        self.nc.clear_and_free_semaphores(list(self.sems.allocated().values()))
        self.nc.all_engine_barrier()


def _emit(nc, tc, ctx_pools):
    """Emit the per-core program. All per-core differences come from inputs."""
    xb16 = nc.declare_dram_parameter("xb16", [KT, P, L], BF16, isOutput=False)
    xres = nc.declare_dram_parameter("xres", [P, L], F32, isOutput=False)
    wqkvT = nc.declare_dram_parameter("wqkvT", [KT, P, 384], BF16, isOutput=False)
    bq = nc.declare_dram_parameter("bq", [P, 1], F32, isOutput=False)
    wprojT4 = nc.declare_dram_parameter("wprojT4", [KT, P, P], BF16, isOutput=False)
    gamma_t = nc.declare_dram_parameter("gamma_t", [KT, P, 1], F32, isOutput=False)
    beta_t = nc.declare_dram_parameter("beta_t", [KT, P, 1], F32, isOutput=False)
    gmask = nc.declare_dram_parameter("gmask", [P, GPT], F32, isOutput=False)
    gmaskT = nc.declare_dram_parameter("gmaskT", [GPT, P], F32, isOutput=False)
    cbias = nc.declare_dram_parameter("cbias", [P, 1], F32, isOutput=False)
    ident = nc.declare_dram_parameter("ident", [P, P], BF16, isOutput=False)
    out = nc.declare_dram_parameter("out", [P, L], F32, isOutput=True)

    # ---------------- long-lived pools ----------------
    cpool = ctx_pools.enter_context(tc.tile_pool(name="consts", bufs=1))
    w_t = []
    for kt in range(KT):
        wt = cpool.tile([P, 384], BF16, name=f"w{kt}")
        nc.sync.dma_start(wt[:], wqkvT[kt])
        w_t.append(wt)
    wp_t = []
    for kt in range(KT):
        wp = cpool.tile([P, P], BF16, name=f"wp{kt}")
        nc.sync.dma_start(wp[:], wprojT4[kt])
        wp_t.append(wp)
    bq_t = cpool.tile([P, 1], F32, name="bqt")
    nc.sync.dma_start(bq_t[:], bq[:])
    gm_t = cpool.tile([P, GPT], F32, name="gmt")
    nc.sync.dma_start(gm_t[:], gmask[:])
    gmT_t = cpool.tile([GPT, P], F32, name="gmTt")
    nc.sync.dma_start(gmT_t[:], gmaskT[:])
    cb_t = cpool.tile([P, 1], F32, name="cbt")
    nc.sync.dma_start(cb_t[:], cbias[:])
    eps_t = cpool.tile([GPT, 1], F32, name="epst")
    nc.gpsimd.memset(eps_t[:], EPS)
    ident_t = cpool.tile([P, P], BF16, name="identt")
    nc.sync.dma_start(ident_t[:], ident[:])
    ga_t, be_t = [], []
    for kt in range(KT):
        g = cpool.tile([P, 1], F32, name=f"ga{kt}")
        nc.sync.dma_start(g[:], gamma_t[kt])
        ga_t.append(g)
        b = cpool.tile([P, 1], F32, name=f"be{kt}")
        nc.sync.dma_start(b[:], beta_t[kt])
        be_t.append(b)

    qkpool = ctx_pools.enter_context(tc.tile_pool(name="qk", bufs=1))
    q_both = qkpool.tile([P, L], BF16, name="q_both")
    k_both = qkpool.tile([P, L], BF16, name="k_both")
    a_both = qkpool.tile([P, L], BF16, name="a_both")
    xres_t = qkpool.tile([P, L], F32, name="xres_t")
    nc.sync.dma_start(xres_t[:], xres[:])

    vtpool = ctx_pools.enter_context(tc.tile_pool(name="vt", bufs=1))
    # [v_h0 (64) | ones | v_h1 (64) | ones]
    vt_t = [vtpool.tile([P, 130], BF16, name=f"vt{si}") for si in range(NS)]
    xnpool = ctx_pools.enter_context(tc.tile_pool(name="xn", bufs=1))
    xn_t = [xnpool.tile([P, L], BF16, name=f"xn{kt}") for kt in range(KT)]

    # prime the ACT exp table set before the main loop needs it
    prim = cpool.tile([1, 2], F32, name="prim")
    nc.gpsimd.memset(prim[:], 0.0)
    nc.scalar.activation(prim[:], prim[:], AF.Exp)

    # ---------------- phase 1: load x, GroupNorm, QKV, vT ----------------
    with tc.tile_pool(name="ph1", bufs=1) as ph1, \
         tc.tile_pool(name="ph1ps", bufs=2, space="PSUM") as ph1ps:
        for kt in range(KT):
            x_t = ph1.tile([P, L], BF16, name=f"x{kt}", tag="x", bufs=4)
            nc.sync.dma_start(x_t[:], xb16[kt])
            sums = ph1.tile([P, 2], F32, name=f"sums{kt}", tag="sums", bufs=4)
            if kt < 2:
                # ScalarE path: identity/square with free-dim accumulate
                scr = ph1.tile([P, L], BF16, name=f"scr{kt}", tag="scr", bufs=2)
                nc.scalar.activation(scr[:], x_t[:], AF.Identity,
                                     accum_out=sums[:, 0:1])
                scr2 = ph1.tile([P, L], BF16, name=f"scr2_{kt}", tag="scr2",
                                bufs=2)
                nc.scalar.activation(scr2[:], x_t[:], AF.Square,
                                     accum_out=sums[:, 1:2])
            else:
                # VectorE path
                nc.vector.tensor_reduce(sums[:, 0:1], x_t[:], AX, ALU.add)
                sq = ph1.tile([P, L], BF16, name=f"sq{kt}", tag="scr2", bufs=2)
                nc.vector.tensor_tensor(sq[:], x_t[:], x_t[:], op=ALU.mult)
                nc.vector.tensor_reduce(sums[:, 1:2], sq[:], AX, ALU.add)
            # group stats: [8, 2] = mask^T @ sums  -> [sum_x, sum_x2] per group
            gs_ps = ph1ps.tile([GPT, 2], F32, name=f"gs{kt}", tag="gs")
            nc.tensor.matmul(gs_ps[:], gm_t[:], sums[:], start=True, stop=True)
            gsm = ph1.tile([GPT, 2], F32, name=f"gsm{kt}", tag="gsm", bufs=2)
            nc.vector.tensor_scalar_mul(gsm[:], gs_ps[:], 1.0 / (CPG * L))
            var = ph1.tile([GPT, 1], F32, name=f"var{kt}", tag="var", bufs=2)
            nc.vector.tensor_tensor(var[:], gsm[:, 0:1], gsm[:, 0:1], op=ALU.mult)
            nc.vector.tensor_tensor(var[:], gsm[:, 1:2], var[:], op=ALU.subtract)
            sd = ph1.tile([GPT, 1], F32, name=f"sd{kt}", tag="sd", bufs=2)
            nc.scalar.activation(sd[:], var[:], AF.Sqrt, bias=eps_t[:])
            grp = ph1.tile([GPT, 2], F32, name=f"grp{kt}", tag="grp", bufs=2)
            nc.vector.reciprocal(grp[:, 0:1], sd[:])
            nc.vector.tensor_copy(grp[:, 1:2], gsm[:, 0:1])
            # expand group -> per-partition (rstd, mean)
            pp_ps = ph1ps.tile([P, 2], F32, name=f"pp{kt}", tag="pp")
            nc.tensor.matmul(pp_ps[:], gmT_t[:], grp[:], start=True, stop=True)
            A = ph1.tile([P, 1], F32, name=f"A{kt}", tag="A", bufs=2)
            nc.vector.tensor_tensor(A[:], pp_ps[:, 0:1], ga_t[kt][:], op=ALU.mult)
            Bt = ph1.tile([P, 1], F32, name=f"B{kt}", tag="B", bufs=2)
            nc.vector.tensor_tensor(Bt[:], pp_ps[:, 1:2], A[:], op=ALU.mult)
            nc.vector.tensor_tensor(Bt[:], be_t[kt][:], Bt[:], op=ALU.subtract)
            if kt < 2:
                nc.scalar.activation(xn_t[kt][:], x_t[:], AF.Identity,
                                     bias=Bt[:], scale=A[:])
            else:
                nc.vector.tensor_scalar(xn_t[kt][:], x_t[:], A[:], Bt[:],
                                        op0=ALU.mult, op1=ALU.add)

        # QKV: k and v first (with vT transposes interleaved), then q chunk 0;
        # q chunks 1..7 are produced inside the main loop (pj-tagged PSUM).
        with tc.tile_pool(name="qkvps", bufs=2, space="PSUM") as qkvps:
            v_both = ph1.tile([P, L], BF16, name="v_both")
            for j, dst in ((1, k_both), (2, v_both)):
                for t in range(NT):
                    ps = qkvps.tile([P, 512], F32, name=f"qkv{j}_{t}",
                                    tag="qkvps")
                    for kt in range(KT):
                        nc.tensor.matmul(
                            ps[:],
                            w_t[kt][:, j * P:(j + 1) * P],
                            xn_t[kt][:, t * 512:(t + 1) * 512],
                            start=(kt == 0), stop=(kt == KT - 1))
                    nc.vector.tensor_copy(
                        dst[:, t * 512:(t + 1) * 512], ps[:])
                    if j == 2:
                        for si in range(4 * t, 4 * t + 4):
                            vps = qkvps.tile([P, P], BF16, name=f"vps{si}",
                                             tag="vps")
                            nc.tensor.transpose(
                                vps[:], v_both[:, si * P:(si + 1) * P],
                                ident_t[:])
                            vt = vt_t[si]
                            nc.vector.memset(vt[:, 64:65], 1.0)
                            nc.vector.memset(vt[:, 129:130], 1.0)
                            nc.vector.tensor_copy(vt[:, 0:64], vps[:, 0:64])
                            nc.scalar.copy(vt[:, 65:129], vps[:, 64:128])
            ps = qkvps.tile([P, 512], F32, name="qkv0_0", tag="qkvps")
            for kt in range(KT):
                nc.tensor.matmul(ps[:], w_t[kt][:, 0:P],
                                 xn_t[kt][:, 0:512],
                                 start=(kt == 0), stop=(kt == KT - 1))
            nc.scalar.activation(q_both[:, 0:512], ps[:], AF.Identity,
                                 bias=bq_t[:])

    # ------- phase 2: pipelined QK -> exp -> AV, fused proj/AG/residual -----
    rgroups = [[0, 1, 2, 3], [4, 5, 6, 7]]
    N = NT * NS
    e_tiles = {}
    av_tiles = {}
    with tc.tile_pool(name="epool", bufs=1) as epool, \
         tc.tile_pool(name="qkps", bufs=2, space="PSUM") as qkps, \
         tc.tile_pool(name="avps", bufs=1, space="PSUM") as avps, \
         tc.tile_pool(name="pjps", bufs=1, space="PSUM") as pjps, \
         tc.tile_pool(name="stg", bufs=1) as stg, \
         tc.tile_pool(name="dram", bufs=1, space="DRAM") as dpool:
        ag_in = [dpool.tile([P, TB], BF16, name=f"agi{tb}") for tb in range(NT)]
        ag_out = [dpool.tile([C, TB], BF16, name=f"ago{tb}") for tb in range(NT)]
        csd = {(tb, h): dpool.tile([1, TB], F32, name=f"csd{tb}_{h}")
               for tb in range(NT) for h in range(2)}

        def emit_qk(g):
            tb, si = divmod(g, NS)
            tsl = slice(tb * TB, (tb + 1) * TB)
            qk = qkps.tile([P, 1024], F32, name=f"qk{g}", tag="qk")
            for h in range(2):
                nc.tensor.matmul(
                    qk[:, h * 512:(h + 1) * 512],
                    k_both[64 * h:64 * h + 64, si * P:(si + 1) * P],
                    q_both[64 * h:64 * h + 64, tsl],
                    start=True, stop=True)
            e_t = epool.tile([P, 1024], BF16, name=f"e{g}", tag="e", bufs=8)
            if FEXP_DVE and g % 2 == 1:
                # whole-tile bitcast only (offset-0): sliced bitcast hangs HW
                nc.vector.tensor_scalar(e_t[:].bitcast(I16), qk[:],
                                        EXPA, EXPB,
                                        op0=ALU.mult, op1=ALU.add)
            else:
                nc.scalar.activation(e_t[:], qk[:], AF.Exp)
            e_tiles[g] = e_t

        def emit_av(g):
            tb, si = divmod(g, NS)
            if si == 0:
                av_tiles[tb] = [
                    avps.tile([65, 512], F32, name=f"av{h}_{tb}", tag=f"av{h}")
                    for h in range(2)]
            e_t = e_tiles.pop(g)
            for h in range(2):
                nc.tensor.matmul(
                    av_tiles[tb][h][:],
                    vt_t[si][:, 65 * h:65 * h + 65],
                    e_t[:, h * 512:(h + 1) * 512],
                    start=(si == 0), stop=(si == NS - 1))

        def emit_norm(tb):
            tsl = slice(tb * TB, (tb + 1) * TB)
            av = av_tiles.pop(tb)
            # copy av out on ScalarE first: releases the PSUM bank for the
            # next t-block's AV chain without waiting on the whole norm chain
            avc = []
            for h in range(2):
                c = stg.tile([65, TB], F32, name=f"avc{tb}_{h}",
                             tag=f"avc{h}", bufs=2)
                nc.scalar.copy(c[:], av[h][:])
                avc.append(c)
            for h in range(2):
                # 1/d on ScalarE: exp(-ln(d)); d in [~50, ~4000] so fp32-safe
                lnd = stg.tile([1, TB], F32, name=f"lnd{tb}_{h}",
                               tag=f"lnd{h}", bufs=2)
                nc.scalar.activation(lnd[:], avc[h][64:65, :], AF.Ln)
                rec = stg.tile([1, TB], F32, name=f"rec{tb}_{h}",
                               tag=f"rec{h}", bufs=2)
                nc.scalar.activation(rec[:], lnd[:], AF.Exp, scale=-1.0)
                nc.sync.dma_start(csd[(tb, h)][:, :], rec[:])
                rb = stg.tile([64, TB], F32, name=f"rb{tb}_{h}",
                              tag=f"rb{h}", bufs=2)
                nc.sync.dma_start(rb[:],
                                  csd[(tb, h)][0:1, :].to_broadcast([64, TB]))
                nc.vector.tensor_tensor(
                    a_both[64 * h:64 * h + 64, tsl], avc[h][0:64, :], rb[:],
                    op=ALU.mult)
            nc.sync.dma_start(ag_in[tb][:, :], a_both[:, tsl])
            nc.gpsimd.collective_compute(
                "AllGather", ALU.bypass, replica_groups=rgroups,
                ins=[ag_in[tb][:, :]], outs=[ag_out[tb][:, :]])

        def emit_proj(tb):
            tsl = slice(tb * TB, (tb + 1) * TB)
            ag_sb = stg.tile([P, KT, TB], BF16, name=f"agsb{tb}", tag="agsb",
                             bufs=2)
            for kt in range(KT):
                nc.gpsimd.dma_start(ag_sb[:, kt, :],
                                    ag_out[tb][kt * P:(kt + 1) * P, :])
            pj = pjps.tile([P, TB], F32, name=f"pj{tb}", tag="pj", bufs=2)
            for kt in range(KT):
                nc.tensor.matmul(pj[:], wp_t[kt][:], ag_sb[:, kt, :],
                                 start=(kt == 0), stop=(kt == KT - 1))
            ob = stg.tile([P, TB], F32, name=f"ob{tb}", tag="ob", bufs=2)
            nc.vector.tensor_tensor(ob[:], pj[:], xres_t[:, tsl], op=ALU.add)
            nc.vector.tensor_scalar_add(ob[:], ob[:], cb_t[:])
            nc.gpsimd.dma_start(out[:, tsl], ob[:])

        def emit_qchunk(tb):
            # q chunk for t-block tb, using a pj-tagged PSUM buffer
            ps = pjps.tile([P, TB], F32, name=f"qch{tb}", tag="pj", bufs=2)
            for kt in range(KT):
                nc.tensor.matmul(ps[:], w_t[kt][:, 0:P],
                                 xn_t[kt][:, tb * TB:(tb + 1) * TB],
                                 start=(kt == 0), stop=(kt == KT - 1))
            nc.scalar.activation(q_both[:, tb * TB:(tb + 1) * TB], ps[:],
                                 AF.Identity, bias=bq_t[:])

        for g in range(N + NS + PJ_OFS + 1):
            tb_g, si_g = divmod(g, NS)
            if si_g == 6 and tb_g + 1 < NT:
                emit_qchunk(tb_g + 1)
            if g < N:
                emit_qk(g)
            h = g - LAG
            if 0 <= h < N:
                emit_av(h)
                if h % NS == NS - 1:
                    emit_norm(h // NS)
            tb_pj, si_pj = divmod(g - PJ_OFS, NS)
            if si_pj == 0 and 1 <= tb_pj <= NT:
                emit_proj(tb_pj - 1)


def _split_waits(nc, limit=1):
    """This toolchain's walrus only encodes `limit` sync waits per
    instruction; hoist excess waits onto same-engine NOPs inserted just
    before the over-limit instruction (semantically a stricter stall)."""
    n_split = 0
    for f in nc.m.functions:
        for bb in f.blocks:
            live = bb.instructions
            new_list = []
            changed = False
            for inst in live:
                si = inst.sync_info
                if si is not None and len(si.on_wait) > limit:
                    waits = list(si.on_wait)
                    extra, keep = waits[:-limit], waits[-limit:]
                    for j in range(0, len(extra), limit):
                        nop = mybir.InstNoOp(
                            name=f"I-wsplit-{nc.next_id()}", ins=[], outs=[])
                        nop.engine = inst.engine
                        nop.sync_info = mybir.SyncInfo(
                            on_wait=extra[j:j + limit], on_update=[])
                        new_list.append(nop)
                        n_split += 1
                    inst.sync_info = mybir.SyncInfo(
                        on_wait=keep, on_update=list(si.on_update))
                    changed = True
                new_list.append(inst)
            if changed:
                live.clear()
                live.extend(new_list)
    return n_split


_CACHE = {}


def _build():
    key = "nc"
    if key not in _CACHE:
        from contextlib import ExitStack
        nc = bass.Bass("TRN2", target_bir_lowering=False, debug=False,
                       num_devices=8)
        with SplitDrainTileContext(nc) as tc:
            with ExitStack() as pools:
                _emit(nc, tc, pools)
        _split_waits(nc)
        _CACHE[key] = nc
    return _CACHE[key]


def _host_inputs(x, gamma, beta, w_qkv, b_qkv, w_proj, b_proj):
    """Build the 8 per-core input maps."""
    xr = x.reshape(B, C, L)
    b_v = np.concatenate([b_qkv[192 * h + 128:192 * h + 192]
                          for h in range(HEADS)])
    cbias_full = w_proj @ b_v + b_proj  # [C]
    gmask = np.zeros((P, GPT), np.float32)
    for p in range(P):
        gmask[p, p // CPG] = 1.0
    xb16 = [np.ascontiguousarray(xr[b].reshape(KT, P, L)).astype(
        ml_dtypes.bfloat16) for b in range(B)]
    in_maps = []
    for core in range(8):
        b, r = divmod(core, 4)
        h0, h1 = 2 * r, 2 * r + 1
        qrows = np.concatenate([192 * h + np.arange(CH) for h in (h0, h1)])
        krows = qrows + CH
        vrows = krows + CH
        wsel = np.concatenate([w_qkv[qrows] * SCALE, w_qkv[krows] * SCALE,
                               w_qkv[vrows]], axis=0)  # [384, C]
        wpT = np.ascontiguousarray(
            w_proj[r * P:(r + 1) * P, :].T).reshape(KT, P, P)
        in_maps.append({
            "xb16": xb16[b],
            "xres": np.ascontiguousarray(xr[b, r * P:(r + 1) * P]),
            "wqkvT": np.ascontiguousarray(
                wsel.T.reshape(KT, P, 384)).astype(ml_dtypes.bfloat16),
            "bq": np.ascontiguousarray(
                (b_qkv[qrows] * SCALE).reshape(P, 1)),
            "wprojT4": wpT.astype(ml_dtypes.bfloat16),
            "gamma_t": np.ascontiguousarray(gamma.reshape(KT, P, 1)),
            "beta_t": np.ascontiguousarray(beta.reshape(KT, P, 1)),
            "gmask": gmask,
            "gmaskT": np.ascontiguousarray(gmask.T),
            "cbias": np.ascontiguousarray(
                cbias_full[r * P:(r + 1) * P].reshape(P, 1)),
            "ident": np.eye(P, dtype=ml_dtypes.bfloat16),
        })
    return in_maps


def kernel(x, gamma, beta, w_qkv, b_qkv, w_proj, b_proj, _trace=False):
    x = np.asarray(x, np.float32)
    gamma = np.asarray(gamma, np.float32)
    beta = np.asarray(beta, np.float32)
    w_qkv = np.asarray(w_qkv, np.float32)
    b_qkv = np.asarray(b_qkv, np.float32)
    w_proj = np.asarray(w_proj, np.float32)
    b_proj = np.asarray(b_proj, np.float32)

    nc = _build()
    in_maps = _host_inputs(x, gamma, beta, w_qkv, b_qkv, w_proj, b_proj)
    res = run_bass_kernel_spmd(nc, in_maps, list(range(8)), trace=_trace)
    out = np.empty((B, C, L), np.float32)
    for core in range(8):
        b, r = divmod(core, 4)
        out[b, r * P:(r + 1) * P] = res.results[core]["out"]
    if _trace:
        kernel.last_results = res
    return out.reshape(B, C, H, W)


# revision 26
# speedup vs baseline: 1.1327x; 1.0620x over previous
"""AttentionBlock (GroupNorm + QKV 1x1 + 8-head attention + proj + residual)
as a Bass/Tile SPMD kernel for 8 Trainium2 NeuronCores.

Sharding: b*heads = 16 attention heads over 8 cores -> 2 heads/core.
GroupNorm + QKV input work is replicated within each 4-core batch group;
the attention outputs `a` are AllGathered (bf16) across the group and each
core computes the output projection for its own 128 output channels.

v2 design notes (vs the ReduceScatter baseline; 546us -> 508us):
 - per-si software pipeline: QK pair (row-tiled) -> exp -> AV, emitted
   interleaved so the PE is never parked waiting on exp and the HAM clock
   gate mostly stays at 8/8.
 - exp alternates whole-si between engines: even si on ScalarE (ACT Exp
   over the [128,1024] PSUM pair), odd si on VectorE via the Schraudolph
   bit trick (y=x*a+b written as int16, reinterpreted as bf16; ~691ns per
   [128,512] at DVE 1x).  End-to-end rel err 5.7e-4 (tolerance 2e-2);
   logits span only [-6.7, 7.1].  NOTE: the int16 bitcast MUST be a
   whole-tile (offset-0) bitcast - a sliced bitcast hangs the hardware.
 - AV stationary is [v | ones] (65 cols); the ones column accumulates the
   softmax denominator in PSUM row 64.  1/d via ACT Ln then Exp(-x)
   (~1e-6 accurate), broadcast across partitions via a DRAM round-trip.
   av PSUM banks are released early by an ACT copy so the next t-block's
   AV chain is not gated on the whole normalize chain.
 - output projection: AllGather of bf16 `a` (4-core groups, ~5-8us each,
   overlapped), then each core computes its 128 output channels locally.
 - q chunks for t-blocks 1..7 are computed inside the main loop.
 - measured limiter: the PE runs ~2048 cycles/si (QK "concurrent"
   row-tiles share the moving-operand XBUS so the pair still costs
   2x512 cycles; AV's two 65-col passes cost another 2x512).  Next
   level would be col-tiled 64|64 AV (one 512-cycle pass) with softmax
   denominators via gpsimd partition-reduce (axis=C) of the e tiles.

Math rewrites (exact softmax invariances, as baseline):
 - k-bias dropped; v-bias + proj-bias folded into host cbias
 - attention scale folded into q-side weights/bias
 - no max-subtraction in softmax (logits are O(+-7), fp32-safe)
"""

import math
import os

os.environ.setdefault("JAX_PLATFORMS", "")

import ml_dtypes
import numpy as np

import concourse.bass as bass
import concourse.mybir as mybir
import concourse.tile as tile
from concourse.bass_utils import run_bass_kernel_spmd
from concourse.vector_clock import ScopedClock

F32 = mybir.dt.float32
BF16 = mybir.dt.bfloat16
I16 = mybir.dt.int16
AX = mybir.AxisListType.X
ALU = mybir.AluOpType
AF = mybir.ActivationFunctionType

B, C, H, W = 2, 512, 64, 64
L = H * W                  # 4096
HEADS = 8
CH = C // HEADS            # 64
GROUPS = 32
GPT = GROUPS // 4          # groups per 128-channel tile = 8
CPG = C // GROUPS          # channels per group = 16
EPS = 1e-6
SCALE = 1.0 / math.sqrt(math.sqrt(CH))

P = 128
KT = C // P                # 4 contraction tiles
NT = L // 512              # 8 t-blocks
NS = L // P                # 32 s-tiles
TB = 512

LAG = 2                    # AV trails QK by LAG si-steps
PJ_OFS = 26                # proj(tb) emitted at step (tb+1)*NS + PJ_OFS
                           # (late enough that the AllGather has completed;
                           # an early proj stalls the in-order PE queue)

# Schraudolph fast-exp for bf16 via int16 bits: bits = x*128/ln2 + (127*128-c)
EXPA = 128.0 / math.log(2.0)
EXPB = 127.0 * 128.0 - 5.5
FEXP_DVE = os.environ.get("FEXP_DVE", "1") == "1"


class SplitDrainTileContext(tile.TileContext):
    """TileContext whose final drain is split into single-wait drains (this
    toolchain's walrus rejects CTRL instructions with >1 sync wait)."""

    def _drain_and_barrier(self, tick_clock, wait_clock):
        g = tick_clock.global_clock
        entries = [(p, g[p]) for p in range(len(g)) if g[p] > 0]
        for proc, tick in entries:
            partial = ScopedClock()
            partial.require_at_least(None, proc, tick)
            d = self.nc.sync.drain()
            wait_clock.add_sem_waits(d.ins, partial)
        if not entries:
            d = self.nc.sync.drain()
            wait_clock.add_sem_waits(d.ins, ScopedClock({None: g}))
        self.nc.all_engine_barrier()
        assert self.sems is not None
        popped = self.nc._tile_sem_poison_stack.pop()
        assert popped is self._sem_poison
        self.nc.clear_and_free_semaphores(list(self.sems.allocated().values()))
        self.nc.all_engine_barrier()


def _emit(nc, tc, ctx_pools):
    """Emit the per-core program. All per-core differences come from inputs."""
    xb16 = nc.declare_dram_parameter("xb16", [KT, P, L], BF16, isOutput=False)
    xres = nc.declare_dram_parameter("xres", [P, L], F32, isOutput=False)
    wqkvT = nc.declare_dram_parameter("wqkvT", [KT, P, 384], BF16, isOutput=False)
    bq = nc.declare_dram_parameter("bq", [P, 1], F32, isOutput=False)
    wprojT4 = nc.declare_dram_parameter("wprojT4", [KT, P, P], BF16, isOutput=False)
    gamma_t = nc.declare_dram_parameter("gamma_t", [KT, P, 1], F32, isOutput=False)
    beta_t = nc.declare_dram_parameter("beta_t", [KT, P, 1], F32, isOutput=False)
    gmask = nc.declare_dram_parameter("gmask", [P, GPT], F32, isOutput=False)
    gmaskT = nc.declare_dram_parameter("gmaskT", [GPT, P], F32, isOutput=False)
    cbias = nc.declare_dram_parameter("cbias", [P, 1], F32, isOutput=False)
    ident = nc.declare_dram_parameter("ident", [P, P], BF16, isOutput=False)
    out = nc.declare_dram_parameter("out", [P, L], F32, isOutput=True)

    # ---------------- long-lived pools ----------------
    cpool = ctx_pools.enter_context(tc.tile_pool(name="consts", bufs=1))
    w_t = []
    for kt in range(KT):
        wt = cpool.tile([P, 384], BF16, name=f"w{kt}")
        nc.sync.dma_start(wt[:], wqkvT[kt])
        w_t.append(wt)
    wp_t = []
    for kt in range(KT):
        wp = cpool.tile([P, P], BF16, name=f"wp{kt}")
        nc.sync.dma_start(wp[:], wprojT4[kt])
        wp_t.append(wp)
    bq_t = cpool.tile([P, 1], F32, name="bqt")
    nc.sync.dma_start(bq_t[:], bq[:])
    gm_t = cpool.tile([P, GPT], F32, name="gmt")
    nc.sync.dma_start(gm_t[:], gmask[:])
    gmT_t = cpool.tile([GPT, P], F32, name="gmTt")
    nc.sync.dma_start(gmT_t[:], gmaskT[:])
    cb_t = cpool.tile([P, 1], F32, name="cbt")
    nc.sync.dma_start(cb_t[:], cbias[:])
    eps_t = cpool.tile([GPT, 1], F32, name="epst")
    nc.gpsimd.memset(eps_t[:], EPS)
    ident_t = cpool.tile([P, P], BF16, name="identt")
    nc.sync.dma_start(ident_t[:], ident[:])
    ga_t, be_t = [], []
    for kt in range(KT):
        g = cpool.tile([P, 1], F32, name=f"ga{kt}")
        nc.sync.dma_start(g[:], gamma_t[kt])
        ga_t.append(g)
        b = cpool.tile([P, 1], F32, name=f"be{kt}")
        nc.sync.dma_start(b[:], beta_t[kt])
        be_t.append(b)

    qkpool = ctx_pools.enter_context(tc.tile_pool(name="qk", bufs=1))
    q_both = qkpool.tile([P, L], BF16, name="q_both")
    k_both = qkpool.tile([P, L], BF16, name="k_both")
    a_both = qkpool.tile([P, L], BF16, name="a_both")
    xres_t = qkpool.tile([P, L], F32, name="xres_t")
    nc.sync.dma_start(xres_t[:], xres[:])

    vtpool = ctx_pools.enter_context(tc.tile_pool(name="vt", bufs=1))
    # [v_h0 (64) | ones | v_h1 (64) | ones]
    vt_t = [vtpool.tile([P, 130], BF16, name=f"vt{si}") for si in range(NS)]
    xnpool = ctx_pools.enter_context(tc.tile_pool(name="xn", bufs=1))
    xn_t = [xnpool.tile([P, L], BF16, name=f"xn{kt}") for kt in range(KT)]

    # prime the ACT exp table set before the main loop needs it
    prim = cpool.tile([1, 2], F32, name="prim")
    nc.gpsimd.memset(prim[:], 0.0)
    nc.scalar.activation(prim[:], prim[:], AF.Exp)

    # ---------------- phase 1: load x, GroupNorm, QKV, vT ----------------
    with tc.tile_pool(name="ph1", bufs=1) as ph1, \
         tc.tile_pool(name="ph1ps", bufs=2, space="PSUM") as ph1ps:
        for kt in range(KT):
            x_t = ph1.tile([P, L], BF16, name=f"x{kt}", tag="x", bufs=4)
            nc.sync.dma_start(x_t[:], xb16[kt])
            sums = ph1.tile([P, 2], F32, name=f"sums{kt}", tag="sums", bufs=4)
            # ScalarE: identity/square with free-dim accumulate (DVE's
            # tensor_reduce is 4.4us/tile here -- keep stats off it)
            scr = ph1.tile([P, L], BF16, name=f"scr{kt}", tag="scr", bufs=2)
            nc.scalar.activation(scr[:], x_t[:], AF.Identity,
                                 accum_out=sums[:, 0:1])
            scr2 = ph1.tile([P, L], BF16, name=f"scr2_{kt}", tag="scr2",
                            bufs=2)
            nc.scalar.activation(scr2[:], x_t[:], AF.Square,
                                 accum_out=sums[:, 1:2])
            # group stats: [8, 2] = mask^T @ sums  -> [sum_x, sum_x2] per group
            gs_ps = ph1ps.tile([GPT, 2], F32, name=f"gs{kt}", tag="gs")
            nc.tensor.matmul(gs_ps[:], gm_t[:], sums[:], start=True, stop=True)
            gsm = ph1.tile([GPT, 2], F32, name=f"gsm{kt}", tag="gsm", bufs=2)
            nc.vector.tensor_scalar_mul(gsm[:], gs_ps[:], 1.0 / (CPG * L))
            var = ph1.tile([GPT, 1], F32, name=f"var{kt}", tag="var", bufs=2)
            nc.vector.tensor_tensor(var[:], gsm[:, 0:1], gsm[:, 0:1], op=ALU.mult)
            nc.vector.tensor_tensor(var[:], gsm[:, 1:2], var[:], op=ALU.subtract)
            sd = ph1.tile([GPT, 1], F32, name=f"sd{kt}", tag="sd", bufs=2)
            nc.scalar.activation(sd[:], var[:], AF.Sqrt, bias=eps_t[:])
            grp = ph1.tile([GPT, 2], F32, name=f"grp{kt}", tag="grp", bufs=2)
            nc.vector.reciprocal(grp[:, 0:1], sd[:])
            nc.vector.tensor_copy(grp[:, 1:2], gsm[:, 0:1])
            # expand group -> per-partition (rstd, mean)
            pp_ps = ph1ps.tile([P, 2], F32, name=f"pp{kt}", tag="pp")
            nc.tensor.matmul(pp_ps[:], gmT_t[:], grp[:], start=True, stop=True)
            A = ph1.tile([P, 1], F32, name=f"A{kt}", tag="A", bufs=2)
            nc.vector.tensor_tensor(A[:], pp_ps[:, 0:1], ga_t[kt][:], op=ALU.mult)
            Bt = ph1.tile([P, 1], F32, name=f"B{kt}", tag="B", bufs=2)
            nc.vector.tensor_tensor(Bt[:], pp_ps[:, 1:2], A[:], op=ALU.mult)
            nc.vector.tensor_tensor(Bt[:], be_t[kt][:], Bt[:], op=ALU.subtract)
            nc.vector.tensor_scalar(xn_t[kt][:], x_t[:], A[:], Bt[:],
                                    op0=ALU.mult, op1=ALU.add)

        # QKV: k and v first (with vT transposes interleaved), then q chunk 0;
        # q chunks 1..7 are produced inside the main loop (pj-tagged PSUM).
        with tc.tile_pool(name="qkvps", bufs=2, space="PSUM") as qkvps:
            v_both = ph1.tile([P, L], BF16, name="v_both")
            for j, dst in ((1, k_both), (2, v_both)):
                for t in range(NT):
                    ps = qkvps.tile([P, 512], F32, name=f"qkv{j}_{t}",
                                    tag="qkvps")
                    for kt in range(KT):
                        nc.tensor.matmul(
                            ps[:],
                            w_t[kt][:, j * P:(j + 1) * P],
                            xn_t[kt][:, t * 512:(t + 1) * 512],
                            start=(kt == 0), stop=(kt == KT - 1))
                    nc.vector.tensor_copy(
                        dst[:, t * 512:(t + 1) * 512], ps[:])
                    if j == 2:
                        for si in range(4 * t, 4 * t + 4):
                            vps = qkvps.tile([P, P], BF16, name=f"vps{si}",
                                             tag="vps")
                            nc.tensor.transpose(
                                vps[:], v_both[:, si * P:(si + 1) * P],
                                ident_t[:])
                            vt = vt_t[si]
                            nc.vector.memset(vt[:, 64:65], 1.0)
                            nc.vector.memset(vt[:, 129:130], 1.0)
                            nc.vector.tensor_copy(vt[:, 0:64], vps[:, 0:64])
                            nc.scalar.copy(vt[:, 65:129], vps[:, 64:128])
            ps = qkvps.tile([P, 512], F32, name="qkv0_0", tag="qkvps")
            for kt in range(KT):
                nc.tensor.matmul(ps[:], w_t[kt][:, 0:P],
                                 xn_t[kt][:, 0:512],
                                 start=(kt == 0), stop=(kt == KT - 1))
            nc.scalar.activation(q_both[:, 0:512], ps[:], AF.Identity,
                                 bias=bq_t[:])

    # ------- phase 2: pipelined QK -> exp -> AV, fused proj/AG/residual -----
    rgroups = [[0, 1, 2, 3], [4, 5, 6, 7]]
    N = NT * NS
    e_tiles = {}
    av_tiles = {}
    with tc.tile_pool(name="epool", bufs=1) as epool, \
         tc.tile_pool(name="qkps", bufs=2, space="PSUM") as qkps, \
         tc.tile_pool(name="avps", bufs=1, space="PSUM") as avps, \
         tc.tile_pool(name="pjps", bufs=1, space="PSUM") as pjps, \
         tc.tile_pool(name="stg", bufs=1) as stg, \
         tc.tile_pool(name="dram", bufs=1, space="DRAM") as dpool:
        ag_in = [dpool.tile([P, TB], BF16, name=f"agi{tb}") for tb in range(NT)]
        ag_out = [dpool.tile([C, TB], BF16, name=f"ago{tb}") for tb in range(NT)]
        csd = {(tb, h): dpool.tile([1, TB], F32, name=f"csd{tb}_{h}")
               for tb in range(NT) for h in range(2)}

        def emit_qk(g):
            tb, si = divmod(g, NS)
            tsl = slice(tb * TB, (tb + 1) * TB)
            qk = qkps.tile([P, 1024], F32, name=f"qk{g}", tag="qk")
            for h in range(2):
                nc.tensor.matmul(
                    qk[:, h * 512:(h + 1) * 512],
                    k_both[64 * h:64 * h + 64, si * P:(si + 1) * P],
                    q_both[64 * h:64 * h + 64, tsl],
                    start=True, stop=True)
            e_t = epool.tile([P, 1024], BF16, name=f"e{g}", tag="e", bufs=10)
            if FEXP_DVE and g % 2 == 1:
                # whole-tile bitcast only (offset-0): sliced bitcast hangs HW
                nc.vector.tensor_scalar(e_t[:].bitcast(I16), qk[:],
                                        EXPA, EXPB,
                                        op0=ALU.mult, op1=ALU.add)
            else:
                nc.scalar.activation(e_t[:], qk[:], AF.Exp)
            e_tiles[g] = e_t

        def emit_av(g):
            tb, si = divmod(g, NS)
            if si == 0:
                av_tiles[tb] = [
                    avps.tile([65, 512], F32, name=f"av{h}_{tb}", tag=f"av{h}")
                    for h in range(2)]
            e_t = e_tiles.pop(g)
            for h in range(2):
                nc.tensor.matmul(
                    av_tiles[tb][h][:],
                    vt_t[si][:, 65 * h:65 * h + 65],
                    e_t[:, h * 512:(h + 1) * 512],
                    start=(si == 0), stop=(si == NS - 1))

        def emit_norm(tb):
            tsl = slice(tb * TB, (tb + 1) * TB)
            av = av_tiles.pop(tb)
            # copy av out on ScalarE first: releases the PSUM bank for the
            # next t-block's AV chain without waiting on the whole norm chain
            avc = []
            for h in range(2):
                c = stg.tile([65, TB], F32, name=f"avc{tb}_{h}",
                             tag=f"avc{h}", bufs=2)
                # on DVE so the ACT exp stream isn't interrupted
                nc.vector.tensor_copy(c[:], av[h][:])
                avc.append(c)
            for h in range(2):
                # 1/d on ScalarE: exp(-ln(d)); d in [~50, ~4000] so fp32-safe
                lnd = stg.tile([1, TB], F32, name=f"lnd{tb}_{h}",
                               tag=f"lnd{h}", bufs=2)
                nc.scalar.activation(lnd[:], avc[h][64:65, :], AF.Ln)
                rec = stg.tile([1, TB], F32, name=f"rec{tb}_{h}",
                               tag=f"rec{h}", bufs=2)
                nc.scalar.activation(rec[:], lnd[:], AF.Exp, scale=-1.0)
                nc.sync.dma_start(csd[(tb, h)][:, :], rec[:])
                rb = stg.tile([64, TB], F32, name=f"rb{tb}_{h}",
                              tag=f"rb{h}", bufs=2)
                nc.sync.dma_start(rb[:],
                                  csd[(tb, h)][0:1, :].to_broadcast([64, TB]))
                nc.vector.tensor_tensor(
                    a_both[64 * h:64 * h + 64, tsl], avc[h][0:64, :], rb[:],
                    op=ALU.mult)
            nc.sync.dma_start(ag_in[tb][:, :], a_both[:, tsl])
            nc.gpsimd.collective_compute(
                "AllGather", ALU.bypass, replica_groups=rgroups,
                ins=[ag_in[tb][:, :]], outs=[ag_out[tb][:, :]])

        def emit_proj(tb):
            tsl = slice(tb * TB, (tb + 1) * TB)
            ag_sb = stg.tile([P, KT, TB], BF16, name=f"agsb{tb}", tag="agsb",
                             bufs=2)
            for kt in range(KT):
                nc.gpsimd.dma_start(ag_sb[:, kt, :],
                                    ag_out[tb][kt * P:(kt + 1) * P, :])
            pj = pjps.tile([P, TB], F32, name=f"pj{tb}", tag="pj", bufs=2)
            for kt in range(KT):
                nc.tensor.matmul(pj[:], wp_t[kt][:], ag_sb[:, kt, :],
                                 start=(kt == 0), stop=(kt == KT - 1))
            ob = stg.tile([P, TB], F32, name=f"ob{tb}", tag="ob", bufs=2)
            nc.vector.tensor_tensor(ob[:], pj[:], xres_t[:, tsl], op=ALU.add)
            nc.vector.tensor_scalar_add(ob[:], ob[:], cb_t[:])
            nc.gpsimd.dma_start(out[:, tsl], ob[:])

        def emit_qchunk(tb):
            # q chunk for t-block tb, using a pj-tagged PSUM buffer
            ps = pjps.tile([P, TB], F32, name=f"qch{tb}", tag="pj", bufs=2)
            for kt in range(KT):
                nc.tensor.matmul(ps[:], w_t[kt][:, 0:P],
                                 xn_t[kt][:, tb * TB:(tb + 1) * TB],
                                 start=(kt == 0), stop=(kt == KT - 1))
            nc.scalar.activation(q_both[:, tb * TB:(tb + 1) * TB], ps[:],
                                 AF.Identity, bias=bq_t[:])

        for g in range(N + NS + PJ_OFS + 1):
            tb_g, si_g = divmod(g, NS)
            if si_g == 6 and tb_g + 1 < NT:
                emit_qchunk(tb_g + 1)
            if g < N:
                emit_qk(g)
            h = g - LAG
            if 0 <= h < N:
                emit_av(h)
                if h % NS == NS - 1:
                    emit_norm(h // NS)
            tb_pj, si_pj = divmod(g - PJ_OFS, NS)
            if si_pj == 0 and 1 <= tb_pj <= NT:
                emit_proj(tb_pj - 1)


def _split_waits(nc, limit=1):
    """This toolchain's walrus only encodes `limit` sync waits per
    instruction; hoist excess waits onto same-engine NOPs inserted just
    before the over-limit instruction (semantically a stricter stall)."""
    n_split = 0
    for f in nc.m.functions:
        for bb in f.blocks:
            live = bb.instructions
            new_list = []
            changed = False
            for inst in live:
                si = inst.sync_info
                if si is not None and len(si.on_wait) > limit:
                    waits = list(si.on_wait)
                    extra, keep = waits[:-limit], waits[-limit:]
                    for j in range(0, len(extra), limit):
                        nop = mybir.InstNoOp(
                            name=f"I-wsplit-{nc.next_id()}", ins=[], outs=[])
                        nop.engine = inst.engine
                        nop.sync_info = mybir.SyncInfo(
                            on_wait=extra[j:j + limit], on_update=[])
                        new_list.append(nop)
                        n_split += 1
                    inst.sync_info = mybir.SyncInfo(
                        on_wait=keep, on_update=list(si.on_update))
                    changed = True
                new_list.append(inst)
            if changed:
                live.clear()
                live.extend(new_list)
    return n_split


_CACHE = {}


def _build():
    key = "nc"
    if key not in _CACHE:
        from contextlib import ExitStack
        nc = bass.Bass("TRN2", target_bir_lowering=False, debug=False,
                       num_devices=8)
        with SplitDrainTileContext(nc) as tc:
            with ExitStack() as pools:
                _emit(nc, tc, pools)
        _split_waits(nc)
        _CACHE[key] = nc
    return _CACHE[key]


def _host_inputs(x, gamma, beta, w_qkv, b_qkv, w_proj, b_proj):
    """Build the 8 per-core input maps."""
    xr = x.reshape(B, C, L)
    b_v = np.concatenate([b_qkv[192 * h + 128:192 * h + 192]
                          for h in range(HEADS)])
    cbias_full = w_proj @ b_v + b_proj  # [C]
    gmask = np.zeros((P, GPT), np.float32)
    for p in range(P):
        gmask[p, p // CPG] = 1.0
    xb16 = [np.ascontiguousarray(xr[b].reshape(KT, P, L)).astype(
        ml_dtypes.bfloat16) for b in range(B)]
    in_maps = []
    for core in range(8):
        b, r = divmod(core, 4)
        h0, h1 = 2 * r, 2 * r + 1
        qrows = np.concatenate([192 * h + np.arange(CH) for h in (h0, h1)])
        krows = qrows + CH
        vrows = krows + CH
        wsel = np.concatenate([w_qkv[qrows] * SCALE, w_qkv[krows] * SCALE,
                               w_qkv[vrows]], axis=0)  # [384, C]
        wpT = np.ascontiguousarray(
            w_proj[r * P:(r + 1) * P, :].T).reshape(KT, P, P)
        in_maps.append({
            "xb16": xb16[b],
            "xres": np.ascontiguousarray(xr[b, r * P:(r + 1) * P]),
            "wqkvT": np.ascontiguousarray(
                wsel.T.reshape(KT, P, 384)).astype(ml_dtypes.bfloat16),
            "bq": np.ascontiguousarray(
                (b_qkv[qrows] * SCALE).reshape(P, 1)),
            "wprojT4": wpT.astype(ml_dtypes.bfloat16),
            "gamma_t": np.ascontiguousarray(gamma.reshape(KT, P, 1)),
            "beta_t": np.ascontiguousarray(beta.reshape(KT, P, 1)),
            "gmask": gmask,
            "gmaskT": np.ascontiguousarray(gmask.T),
            "cbias": np.ascontiguousarray(
                cbias_full[r * P:(r + 1) * P].reshape(P, 1)),
            "ident": np.eye(P, dtype=ml_dtypes.bfloat16),
        })
    return in_maps


def kernel(x, gamma, beta, w_qkv, b_qkv, w_proj, b_proj, _trace=False):
    x = np.asarray(x, np.float32)
    gamma = np.asarray(gamma, np.float32)
    beta = np.asarray(beta, np.float32)
    w_qkv = np.asarray(w_qkv, np.float32)
    b_qkv = np.asarray(b_qkv, np.float32)
    w_proj = np.asarray(w_proj, np.float32)
    b_proj = np.asarray(b_proj, np.float32)

    nc = _build()
    in_maps = _host_inputs(x, gamma, beta, w_qkv, b_qkv, w_proj, b_proj)
    res = run_bass_kernel_spmd(nc, in_maps, list(range(8)), trace=_trace)
    out = np.empty((B, C, L), np.float32)
    for core in range(8):
        b, r = divmod(core, 4)
        out[b, r * P:(r + 1) * P] = res.results[core]["out"]
    if _trace:
        kernel.last_results = res
    return out.reshape(B, C, H, W)


# revision 29
# speedup vs baseline: 1.2492x; 1.1028x over previous
"""AttentionBlock (GroupNorm + QKV 1x1 + 8-head attention + proj + residual)
as a Bass/Tile SPMD kernel for 8 Trainium2 NeuronCores.

Sharding: b*heads = 16 attention heads over 8 cores -> 2 heads/core.
GroupNorm + QKV input work is replicated within each 4-core batch group;
the attention outputs `a` are AllGathered (bf16) across the group and each
core computes the output projection for its own 128 output channels.

v2 design notes (vs the ReduceScatter baseline; 546us -> 508us):
 - per-si software pipeline: QK pair (row-tiled) -> exp -> AV, emitted
   interleaved so the PE is never parked waiting on exp and the HAM clock
   gate mostly stays at 8/8.
 - exp alternates whole-si between engines: even si on ScalarE (ACT Exp
   over the [128,1024] PSUM pair), odd si on VectorE via the Schraudolph
   bit trick (y=x*a+b written as int16, reinterpreted as bf16; ~691ns per
   [128,512] at DVE 1x).  End-to-end rel err 5.7e-4 (tolerance 2e-2);
   logits span only [-6.7, 7.1].  NOTE: the int16 bitcast MUST be a
   whole-tile (offset-0) bitcast - a sliced bitcast hangs the hardware.
 - AV stationary is [v | ones] (65 cols); the ones column accumulates the
   softmax denominator in PSUM row 64.  1/d via ACT Ln then Exp(-x)
   (~1e-6 accurate), broadcast across partitions via a DRAM round-trip.
   av PSUM banks are released early by an ACT copy so the next t-block's
   AV chain is not gated on the whole normalize chain.
 - output projection: AllGather of bf16 `a` (4-core groups, ~5-8us each,
   overlapped), then each core computes its 128 output channels locally.
 - q chunks for t-blocks 1..7 are computed inside the main loop.
 - measured limiter: the PE runs ~2048 cycles/si (QK "concurrent"
   row-tiles share the moving-operand XBUS so the pair still costs
   2x512 cycles; AV's two 65-col passes cost another 2x512).  Next
   level would be col-tiled 64|64 AV (one 512-cycle pass) with softmax
   denominators via gpsimd partition-reduce (axis=C) of the e tiles.

Math rewrites (exact softmax invariances, as baseline):
 - k-bias dropped; v-bias + proj-bias folded into host cbias
 - attention scale folded into q-side weights/bias
 - no max-subtraction in softmax (logits are O(+-7), fp32-safe)
"""

import math
import os

os.environ.setdefault("JAX_PLATFORMS", "")

import ml_dtypes
import numpy as np

import concourse.bass as bass
import concourse.mybir as mybir
import concourse.tile as tile
from concourse.bass_utils import run_bass_kernel_spmd
from concourse.vector_clock import ScopedClock

F32 = mybir.dt.float32
BF16 = mybir.dt.bfloat16
I16 = mybir.dt.int16
AX = mybir.AxisListType.X
ALU = mybir.AluOpType
AF = mybir.ActivationFunctionType

B, C, H, W = 2, 512, 64, 64
L = H * W                  # 4096
HEADS = 8
CH = C // HEADS            # 64
GROUPS = 32
GPT = GROUPS // 4          # groups per 128-channel tile = 8
CPG = C // GROUPS          # channels per group = 16
EPS = 1e-6
SCALE = 1.0 / math.sqrt(math.sqrt(CH))

P = 128
KT = C // P                # 4 contraction tiles
NT = L // 512              # 8 t-blocks
NS = L // P                # 32 s-tiles
TB = 512

LAG = 2                    # AV trails QK by LAG si-steps
PJ_OFS = 26                # proj(tb) emitted at step (tb+1)*NS + PJ_OFS
                           # (late enough that the AllGather has completed;
                           # an early proj stalls the in-order PE queue)

# Schraudolph fast-exp for bf16 via int16 bits: bits = x*128/ln2 + (127*128-c)
EXPA = 128.0 / math.log(2.0)
EXPB = 127.0 * 128.0 - 5.5
FEXP_DVE = os.environ.get("FEXP_DVE", "1") == "1"


class SplitDrainTileContext(tile.TileContext):
    """TileContext whose final drain is split into single-wait drains (this
    toolchain's walrus rejects CTRL instructions with >1 sync wait)."""

    def _drain_and_barrier(self, tick_clock, wait_clock):
        g = tick_clock.global_clock
        entries = [(p, g[p]) for p in range(len(g)) if g[p] > 0]
        for proc, tick in entries:
            partial = ScopedClock()
            partial.require_at_least(None, proc, tick)
            d = self.nc.sync.drain()
            wait_clock.add_sem_waits(d.ins, partial)
        if not entries:
            d = self.nc.sync.drain()
            wait_clock.add_sem_waits(d.ins, ScopedClock({None: g}))
        self.nc.all_engine_barrier()
        assert self.sems is not None
        popped = self.nc._tile_sem_poison_stack.pop()
        assert popped is self._sem_poison
        self.nc.clear_and_free_semaphores(list(self.sems.allocated().values()))
        self.nc.all_engine_barrier()


def _emit(nc, tc, ctx_pools):
    """Emit the per-core program. All per-core differences come from inputs."""
    xb16 = nc.declare_dram_parameter("xb16", [KT, P, L], BF16, isOutput=False)
    xres = nc.declare_dram_parameter("xres", [P, L], F32, isOutput=False)
    wqkvT = nc.declare_dram_parameter("wqkvT", [KT, P, 384], BF16, isOutput=False)
    bq = nc.declare_dram_parameter("bq", [P, 1], F32, isOutput=False)
    wprojT4 = nc.declare_dram_parameter("wprojT4", [KT, P, P], BF16, isOutput=False)
    gamma_t = nc.declare_dram_parameter("gamma_t", [KT, P, 1], F32, isOutput=False)
    beta_t = nc.declare_dram_parameter("beta_t", [KT, P, 1], F32, isOutput=False)
    gmask = nc.declare_dram_parameter("gmask", [P, GPT], F32, isOutput=False)
    gmaskT = nc.declare_dram_parameter("gmaskT", [GPT, P], F32, isOutput=False)
    cbias = nc.declare_dram_parameter("cbias", [P, 1], F32, isOutput=False)
    ident = nc.declare_dram_parameter("ident", [P, P], BF16, isOutput=False)
    out = nc.declare_dram_parameter("out", [P, L], F32, isOutput=True)

    # ---------------- long-lived pools ----------------
    cpool = ctx_pools.enter_context(tc.tile_pool(name="consts", bufs=1))
    w_t = []
    for kt in range(KT):
        wt = cpool.tile([P, 384], BF16, name=f"w{kt}")
        nc.sync.dma_start(wt[:], wqkvT[kt])
        w_t.append(wt)
    wp_t = []
    for kt in range(KT):
        wp = cpool.tile([P, P], BF16, name=f"wp{kt}")
        nc.sync.dma_start(wp[:], wprojT4[kt])
        wp_t.append(wp)
    bq_t = cpool.tile([P, 1], F32, name="bqt")
    nc.sync.dma_start(bq_t[:], bq[:])
    gm_t = cpool.tile([P, GPT], F32, name="gmt")
    nc.sync.dma_start(gm_t[:], gmask[:])
    gmT_t = cpool.tile([GPT, P], F32, name="gmTt")
    nc.sync.dma_start(gmT_t[:], gmaskT[:])
    cb_t = cpool.tile([P, 1], F32, name="cbt")
    nc.sync.dma_start(cb_t[:], cbias[:])
    eps_t = cpool.tile([GPT, 1], F32, name="epst")
    # on VectorE: the gpsimd queue sits behind the cc bootstrap barrier
    nc.vector.memset(eps_t[:], EPS)
    ident_t = cpool.tile([P, P], BF16, name="identt")
    nc.sync.dma_start(ident_t[:], ident[:])
    ga_t, be_t = [], []
    for kt in range(KT):
        g = cpool.tile([P, 1], F32, name=f"ga{kt}")
        nc.sync.dma_start(g[:], gamma_t[kt])
        ga_t.append(g)
        b = cpool.tile([P, 1], F32, name=f"be{kt}")
        nc.sync.dma_start(b[:], beta_t[kt])
        be_t.append(b)

    qkpool = ctx_pools.enter_context(tc.tile_pool(name="qk", bufs=1))
    q_both = qkpool.tile([P, L], BF16, name="q_both")
    k_both = qkpool.tile([P, L], BF16, name="k_both")
    a_both = qkpool.tile([P, L], BF16, name="a_both")
    xres_t = qkpool.tile([P, L], F32, name="xres_t")
    nc.sync.dma_start(xres_t[:], xres[:])

    vtpool = ctx_pools.enter_context(tc.tile_pool(name="vt", bufs=1))
    # [v_h0 (64) | ones | v_h1 (64) | ones]
    vt_t = [vtpool.tile([P, 130], BF16, name=f"vt{si}") for si in range(NS)]
    xnpool = ctx_pools.enter_context(tc.tile_pool(name="xn", bufs=1))
    xn_t = [xnpool.tile([P, L], BF16, name=f"xn{kt}") for kt in range(KT)]

    # prime the ACT exp table set before the main loop needs it
    prim = cpool.tile([1, 2], F32, name="prim")
    nc.vector.memset(prim[:], 0.0)
    nc.scalar.activation(prim[:], prim[:], AF.Exp)

    # warm up the collective ring early so the first real AllGather in the
    # main loop doesn't pay the ~20us first-collective cost
    dpool0 = ctx_pools.enter_context(tc.tile_pool(name="dram0", bufs=1,
                                                  space="DRAM"))
    warm_sb = cpool.tile([1, 64], BF16, name="warm_sb")
    nc.vector.memset(warm_sb[:], 0.0)
    warm_in = dpool0.tile([1, 64], BF16, name="warm_in")
    warm_out = dpool0.tile([4, 64], BF16, name="warm_out")
    nc.sync.dma_start(warm_in[:, :], warm_sb[:])
    nc.gpsimd.collective_compute(
        "AllGather", ALU.bypass, replica_groups=[[0, 1, 2, 3], [4, 5, 6, 7]],
        ins=[warm_in[:, :]], outs=[warm_out[:, :]])

    # ---------------- phase 1: load x, GroupNorm, QKV, vT ----------------
    with tc.tile_pool(name="ph1", bufs=1) as ph1, \
         tc.tile_pool(name="ph1ps", bufs=2, space="PSUM") as ph1ps:
        for kt in range(KT):
            x_t = ph1.tile([P, L], BF16, name=f"x{kt}", tag="x", bufs=4)
            nc.sync.dma_start(x_t[:], xb16[kt])
            sums = ph1.tile([P, 2], F32, name=f"sums{kt}", tag="sums", bufs=4)
            if kt < 3:
                # ScalarE: identity/square with free-dim accumulate
                scr = ph1.tile([P, L], BF16, name=f"scr{kt}", tag="scr",
                               bufs=2)
                nc.scalar.activation(scr[:], x_t[:], AF.Identity,
                                     accum_out=sums[:, 0:1])
                scr2 = ph1.tile([P, L], BF16, name=f"scr2_{kt}", tag="scr2",
                                bufs=2)
                nc.scalar.activation(scr2[:], x_t[:], AF.Square,
                                     accum_out=sums[:, 1:2])
            else:
                # last tile on VectorE, in parallel with ScalarE's kt 0-2
                nc.vector.tensor_reduce(sums[:, 0:1], x_t[:], AX, ALU.add)
                sq = ph1.tile([P, L], BF16, name=f"sq{kt}", tag="scr2",
                              bufs=2)
                nc.vector.tensor_tensor(sq[:], x_t[:], x_t[:], op=ALU.mult)
                nc.vector.tensor_reduce(sums[:, 1:2], sq[:], AX, ALU.add)
            # group stats: [8, 2] = mask^T @ sums  -> [sum_x, sum_x2] per group
            gs_ps = ph1ps.tile([GPT, 2], F32, name=f"gs{kt}", tag="gs")
            nc.tensor.matmul(gs_ps[:], gm_t[:], sums[:], start=True, stop=True)
            gsm = ph1.tile([GPT, 2], F32, name=f"gsm{kt}", tag="gsm", bufs=2)
            nc.vector.tensor_scalar_mul(gsm[:], gs_ps[:], 1.0 / (CPG * L))
            var = ph1.tile([GPT, 1], F32, name=f"var{kt}", tag="var", bufs=2)
            nc.vector.tensor_tensor(var[:], gsm[:, 0:1], gsm[:, 0:1], op=ALU.mult)
            nc.vector.tensor_tensor(var[:], gsm[:, 1:2], var[:], op=ALU.subtract)
            sd = ph1.tile([GPT, 1], F32, name=f"sd{kt}", tag="sd", bufs=2)
            nc.scalar.activation(sd[:], var[:], AF.Sqrt, bias=eps_t[:])
            grp = ph1.tile([GPT, 2], F32, name=f"grp{kt}", tag="grp", bufs=2)
            nc.vector.reciprocal(grp[:, 0:1], sd[:])
            nc.vector.tensor_copy(grp[:, 1:2], gsm[:, 0:1])
            # expand group -> per-partition (rstd, mean)
            pp_ps = ph1ps.tile([P, 2], F32, name=f"pp{kt}", tag="pp")
            nc.tensor.matmul(pp_ps[:], gmT_t[:], grp[:], start=True, stop=True)
            A = ph1.tile([P, 1], F32, name=f"A{kt}", tag="A", bufs=2)
            nc.vector.tensor_tensor(A[:], pp_ps[:, 0:1], ga_t[kt][:], op=ALU.mult)
            Bt = ph1.tile([P, 1], F32, name=f"B{kt}", tag="B", bufs=2)
            nc.vector.tensor_tensor(Bt[:], pp_ps[:, 1:2], A[:], op=ALU.mult)
            nc.vector.tensor_tensor(Bt[:], be_t[kt][:], Bt[:], op=ALU.subtract)
            nc.vector.tensor_scalar(xn_t[kt][:], x_t[:], A[:], Bt[:],
                                    op0=ALU.mult, op1=ALU.add)

        # QKV: k and v first (with vT transposes interleaved), then q chunk 0;
        # q chunks 1..7 are produced inside the main loop (pj-tagged PSUM).
        with tc.tile_pool(name="qkvps", bufs=2, space="PSUM") as qkvps:
            v_both = ph1.tile([P, L], BF16, name="v_both")
            for j, dst in ((1, k_both), (2, v_both)):
                for t in range(NT):
                    ps = qkvps.tile([P, 512], F32, name=f"qkv{j}_{t}",
                                    tag="qkvps")
                    for kt in range(KT):
                        nc.tensor.matmul(
                            ps[:],
                            w_t[kt][:, j * P:(j + 1) * P],
                            xn_t[kt][:, t * 512:(t + 1) * 512],
                            start=(kt == 0), stop=(kt == KT - 1))
                    nc.vector.tensor_copy(
                        dst[:, t * 512:(t + 1) * 512], ps[:])
                    if j == 2:
                        for si in range(4 * t, 4 * t + 4):
                            vps = qkvps.tile([P, P], BF16, name=f"vps{si}",
                                             tag="vps")
                            nc.tensor.transpose(
                                vps[:], v_both[:, si * P:(si + 1) * P],
                                ident_t[:])
                            vt = vt_t[si]
                            nc.vector.memset(vt[:, 64:65], 1.0)
                            nc.vector.memset(vt[:, 129:130], 1.0)
                            nc.vector.tensor_copy(vt[:, 0:64], vps[:, 0:64])
                            nc.scalar.copy(vt[:, 65:129], vps[:, 64:128])
            ps = qkvps.tile([P, 512], F32, name="qkv0_0", tag="qkvps")
            for kt in range(KT):
                nc.tensor.matmul(ps[:], w_t[kt][:, 0:P],
                                 xn_t[kt][:, 0:512],
                                 start=(kt == 0), stop=(kt == KT - 1))
            nc.scalar.activation(q_both[:, 0:512], ps[:], AF.Identity,
                                 bias=bq_t[:])

    # ------- phase 2: pipelined QK -> exp -> AV, fused proj/AG/residual -----
    rgroups = [[0, 1, 2, 3], [4, 5, 6, 7]]
    N = NT * NS
    e_tiles = {}
    av_tiles = {}
    with tc.tile_pool(name="epool", bufs=1) as epool, \
         tc.tile_pool(name="qkps", bufs=2, space="PSUM") as qkps, \
         tc.tile_pool(name="avps", bufs=1, space="PSUM") as avps, \
         tc.tile_pool(name="pjps", bufs=1, space="PSUM") as pjps, \
         tc.tile_pool(name="stg", bufs=1) as stg, \
         tc.tile_pool(name="dram", bufs=1, space="DRAM") as dpool:
        ag_in = [dpool.tile([P, TB], BF16, name=f"agi{tb}") for tb in range(NT)]
        ag_out = [dpool.tile([C, TB], BF16, name=f"ago{tb}") for tb in range(NT)]
        csd = {(tb, h): dpool.tile([1, TB], F32, name=f"csd{tb}_{h}")
               for tb in range(NT) for h in range(2)}

        def emit_qk(g):
            tb, si = divmod(g, NS)
            tsl = slice(tb * TB, (tb + 1) * TB)
            qk = qkps.tile([P, 1024], F32, name=f"qk{g}", tag="qk")
            for h in range(2):
                nc.tensor.matmul(
                    qk[:, h * 512:(h + 1) * 512],
                    k_both[64 * h:64 * h + 64, si * P:(si + 1) * P],
                    q_both[64 * h:64 * h + 64, tsl],
                    start=True, stop=True)
            e_t = epool.tile([P, 1024], BF16, name=f"e{g}", tag="e", bufs=10)
            if FEXP_DVE and g % 2 == 1:
                # whole-tile bitcast only (offset-0): sliced bitcast hangs HW
                nc.vector.tensor_scalar(e_t[:].bitcast(I16), qk[:],
                                        EXPA, EXPB,
                                        op0=ALU.mult, op1=ALU.add)
            else:
                nc.scalar.activation(e_t[:], qk[:], AF.Exp)
            e_tiles[g] = e_t

        def emit_av(g):
            tb, si = divmod(g, NS)
            if si == 0:
                av_tiles[tb] = [
                    avps.tile([65, 512], F32, name=f"av{h}_{tb}", tag=f"av{h}")
                    for h in range(2)]
            e_t = e_tiles.pop(g)
            for h in range(2):
                nc.tensor.matmul(
                    av_tiles[tb][h][:],
                    vt_t[si][:, 65 * h:65 * h + 65],
                    e_t[:, h * 512:(h + 1) * 512],
                    start=(si == 0), stop=(si == NS - 1))

        def emit_norm(tb):
            tsl = slice(tb * TB, (tb + 1) * TB)
            av = av_tiles.pop(tb)
            # copy av out on ScalarE first: releases the PSUM bank for the
            # next t-block's AV chain without waiting on the whole norm chain
            avc = []
            for h in range(2):
                c = stg.tile([65, TB], F32, name=f"avc{tb}_{h}",
                             tag=f"avc{h}", bufs=2)
                # on DVE so the ACT exp stream isn't interrupted
                nc.vector.tensor_copy(c[:], av[h][:])
                avc.append(c)
            for h in range(2):
                # 1/d on ScalarE: exp(-ln(d)); d in [~50, ~4000] so fp32-safe
                lnd = stg.tile([1, TB], F32, name=f"lnd{tb}_{h}",
                               tag=f"lnd{h}", bufs=2)
                nc.scalar.activation(lnd[:], avc[h][64:65, :], AF.Ln)
                rec = stg.tile([1, TB], F32, name=f"rec{tb}_{h}",
                               tag=f"rec{h}", bufs=2)
                nc.scalar.activation(rec[:], lnd[:], AF.Exp, scale=-1.0)
                nc.sync.dma_start(csd[(tb, h)][:, :], rec[:])
                rb = stg.tile([64, TB], F32, name=f"rb{tb}_{h}",
                              tag=f"rb{h}", bufs=2)
                nc.sync.dma_start(rb[:],
                                  csd[(tb, h)][0:1, :].to_broadcast([64, TB]))
                nc.vector.tensor_tensor(
                    a_both[64 * h:64 * h + 64, tsl], avc[h][0:64, :], rb[:],
                    op=ALU.mult)
            nc.sync.dma_start(ag_in[tb][:, :], a_both[:, tsl])
            nc.gpsimd.collective_compute(
                "AllGather", ALU.bypass, replica_groups=rgroups,
                ins=[ag_in[tb][:, :]], outs=[ag_out[tb][:, :]])

        def emit_proj(tb):
            tsl = slice(tb * TB, (tb + 1) * TB)
            ag_sb = stg.tile([P, KT, TB], BF16, name=f"agsb{tb}", tag="agsb",
                             bufs=2)
            for kt in range(KT):
                nc.gpsimd.dma_start(ag_sb[:, kt, :],
                                    ag_out[tb][kt * P:(kt + 1) * P, :])
            pj = pjps.tile([P, TB], F32, name=f"pj{tb}", tag="pj", bufs=2)
            for kt in range(KT):
                nc.tensor.matmul(pj[:], wp_t[kt][:], ag_sb[:, kt, :],
                                 start=(kt == 0), stop=(kt == KT - 1))
            ob = stg.tile([P, TB], F32, name=f"ob{tb}", tag="ob", bufs=2)
            nc.vector.tensor_tensor(ob[:], pj[:], xres_t[:, tsl], op=ALU.add)
            nc.vector.tensor_scalar_add(ob[:], ob[:], cb_t[:])
            nc.gpsimd.dma_start(out[:, tsl], ob[:])

        def emit_qchunk(tb):
            # q chunk for t-block tb, using a pj-tagged PSUM buffer
            ps = pjps.tile([P, TB], F32, name=f"qch{tb}", tag="pj", bufs=2)
            for kt in range(KT):
                nc.tensor.matmul(ps[:], w_t[kt][:, 0:P],
                                 xn_t[kt][:, tb * TB:(tb + 1) * TB],
                                 start=(kt == 0), stop=(kt == KT - 1))
            nc.scalar.activation(q_both[:, tb * TB:(tb + 1) * TB], ps[:],
                                 AF.Identity, bias=bq_t[:])

        for g in range(N + NS + PJ_OFS + 1):
            tb_g, si_g = divmod(g, NS)
            if si_g == 6 and tb_g + 1 < NT:
                emit_qchunk(tb_g + 1)
            if g < N:
                emit_qk(g)
            h = g - LAG
            if 0 <= h < N:
                emit_av(h)
                if h % NS == NS - 1:
                    emit_norm(h // NS)
            tb_pj, si_pj = divmod(g - PJ_OFS, NS)
            if si_pj == 0 and 1 <= tb_pj <= NT:
                emit_proj(tb_pj - 1)


def _split_waits(nc, limit=1):
    """This toolchain's walrus only encodes `limit` sync waits per
    instruction; hoist excess waits onto same-engine NOPs inserted just
    before the over-limit instruction (semantically a stricter stall)."""
    n_split = 0
    for f in nc.m.functions:
        for bb in f.blocks:
            live = bb.instructions
            new_list = []
            changed = False
            for inst in live:
                si = inst.sync_info
                if si is not None and len(si.on_wait) > limit:
                    waits = list(si.on_wait)
                    extra, keep = waits[:-limit], waits[-limit:]
                    for j in range(0, len(extra), limit):
                        nop = mybir.InstNoOp(
                            name=f"I-wsplit-{nc.next_id()}", ins=[], outs=[])
                        nop.engine = inst.engine
                        nop.sync_info = mybir.SyncInfo(
                            on_wait=extra[j:j + limit], on_update=[])
                        new_list.append(nop)
                        n_split += 1
                    inst.sync_info = mybir.SyncInfo(
                        on_wait=keep, on_update=list(si.on_update))
                    changed = True
                new_list.append(inst)
            if changed:
                live.clear()
                live.extend(new_list)
    return n_split


_CACHE = {}


def _build():
    key = "nc"
    if key not in _CACHE:
        from contextlib import ExitStack
        nc = bass.Bass("TRN2", target_bir_lowering=False, debug=False,
                       num_devices=8)
        with SplitDrainTileContext(nc) as tc:
            with ExitStack() as pools:
                _emit(nc, tc, pools)
        _split_waits(nc)
        _CACHE[key] = nc
    return _CACHE[key]


def _host_inputs(x, gamma, beta, w_qkv, b_qkv, w_proj, b_proj):
    """Build the 8 per-core input maps."""
    xr = x.reshape(B, C, L)
    b_v = np.concatenate([b_qkv[192 * h + 128:192 * h + 192]
                          for h in range(HEADS)])
    cbias_full = w_proj @ b_v + b_proj  # [C]
    gmask = np.zeros((P, GPT), np.float32)
    for p in range(P):
        gmask[p, p // CPG] = 1.0
    xb16 = [np.ascontiguousarray(xr[b].reshape(KT, P, L)).astype(
        ml_dtypes.bfloat16) for b in range(B)]
    in_maps = []
    for core in range(8):
        b, r = divmod(core, 4)
        h0, h1 = 2 * r, 2 * r + 1
        qrows = np.concatenate([192 * h + np.arange(CH) for h in (h0, h1)])
        krows = qrows + CH
        vrows = krows + CH
        wsel = np.concatenate([w_qkv[qrows] * SCALE, w_qkv[krows] * SCALE,
                               w_qkv[vrows]], axis=0)  # [384, C]
        wpT = np.ascontiguousarray(
            w_proj[r * P:(r + 1) * P, :].T).reshape(KT, P, P)
        in_maps.append({
            "xb16": xb16[b],
            "xres": np.ascontiguousarray(xr[b, r * P:(r + 1) * P]),
            "wqkvT": np.ascontiguousarray(
                wsel.T.reshape(KT, P, 384)).astype(ml_dtypes.bfloat16),
            "bq": np.ascontiguousarray(
                (b_qkv[qrows] * SCALE).reshape(P, 1)),
            "wprojT4": wpT.astype(ml_dtypes.bfloat16),
            "gamma_t": np.ascontiguousarray(gamma.reshape(KT, P, 1)),
            "beta_t": np.ascontiguousarray(beta.reshape(KT, P, 1)),
            "gmask": gmask,
            "gmaskT": np.ascontiguousarray(gmask.T),
            "cbias": np.ascontiguousarray(
                cbias_full[r * P:(r + 1) * P].reshape(P, 1)),
            "ident": np.eye(P, dtype=ml_dtypes.bfloat16),
        })
    return in_maps


def kernel(x, gamma, beta, w_qkv, b_qkv, w_proj, b_proj, _trace=False):
    x = np.asarray(x, np.float32)
    gamma = np.asarray(gamma, np.float32)
    beta = np.asarray(beta, np.float32)
    w_qkv = np.asarray(w_qkv, np.float32)
    b_qkv = np.asarray(b_qkv, np.float32)
    w_proj = np.asarray(w_proj, np.float32)
    b_proj = np.asarray(b_proj, np.float32)

    nc = _build()
    in_maps = _host_inputs(x, gamma, beta, w_qkv, b_qkv, w_proj, b_proj)
    res = run_bass_kernel_spmd(nc, in_maps, list(range(8)), trace=_trace)
    out = np.empty((B, C, L), np.float32)
    for core in range(8):
        b, r = divmod(core, 4)
        out[b, r * P:(r + 1) * P] = res.results[core]["out"]
    if _trace:
        kernel.last_results = res
    return out.reshape(B, C, H, W)
